# revision 31
# baseline (speedup 1.0000x reference)
"""Trainium2 Bass kernel for nn_Attr_Relation_Net (gnn_message_passing).

Computation per edge e (E = 400000):
    obs_h  = obs_embs[obs_idx[e]]                 # [256] gather
    m_i    = known_mask[obs_mask_idx[e]]          # [64]  gather
    a      = attr_idx[e]
    a_j_i  = G[a]   where G = feature_emb @ feature_emb.T   (64x64, on device)
    m      = m_i with column a zeroed             # m_i * self_mask[a]
    s      = softmax(m) = (1 + (e-1)*m) / (64 + (e-1)*sum(m))   (m in {0,1})
    mJI    = gelu(gelu(s @ rm_W1 + rm_b1) @ rm_W2 + rm_b2)
    h2     = gelu((a_j_i * mJI) @ rr_W + rr_b)
    out[e] = gelu((obs_h * h2) @ rc_W + rc_b)

Sharding: edges are assigned to the 8 cores by obs_idx range (12500 rows of
obs_embs per core, so the obs_embs table is sharded by row and gather indices
fit int16 for the fast bulk-gather ucode). Within a core, edges are bucketed
into 4 obs_mask_idx ranges (<=32768 rows each) occupying fixed slot regions,
so known_mask gathers read a sliced table view with int16-local indices.
The host assembles per-core slot orderings and un-permutes the output.

On-chip layout: all matmul stages run feat-major ([feat, edges]). obs_embs is
stored bf16 in DRAM and bulk-gathered with transpose=True so obs_h arrives
feat-major directly (no PE transposes). The per-edge front end (masking +
closed-form softmax) runs row-major where per-edge broadcasts and reductions
are native; one PE transpose per 128 edges moves [eq | s] into feat-major.
a_j_i = G @ eq on PE (bf16). The final matmul is weight-stationary
(lhsT = rc_W chunks) producing feat-major output; rc_b is folded into the
GELU activation's per-partition bias; the output is written to DRAM as
[HID, ECP] and transposed back on the host. Matmuls run in float32r/bf16
(full PE rate at N>=256).
"""

import numpy as np

try:
    import ml_dtypes
    BF16 = np.dtype(ml_dtypes.bfloat16)
except ImportError:  # pragma: no cover
    BF16 = np.float32

E_TOT = 400000
N_CORES = 8
HID = 256
NF = 64
N_ROWS = 100000
P = 128

RPC = N_ROWS // N_CORES        # obs_embs rows per core (12500)
MASK_BASES = (0, 32768, 65536, 98304, 100000)
CAPQ = (17408, 17408, 17408, 1024)   # per-mask-range slot capacities
ECP = sum(CAPQ)                # 53248 slots per core
CHUNK = 2048                   # obs-gather granularity
MIBLK = 1024                   # known_mask-gather granularity
W = 512                        # compute tile (edges)
N_CHUNKS = ECP // CHUNK        # 26
TILES_PER_CHUNK = CHUNK // W   # 4
NGROUP = W // 128              # 4
NJ = ECP // P                  # 416 wrapped columns
NI16 = ECP // 16               # 3328 idx columns

EM1 = float(np.e - 1.0)

# region of each 1024-slot block
_CUM = np.cumsum((0,) + CAPQ)
BLK_REGION = [int(np.searchsorted(_CUM, b * MIBLK, side="right") - 1)
              for b in range(ECP // MIBLK)]


def build_nc(sim_gelu=False, n_chunks=N_CHUNKS):
    import concourse.bacc as bacc
    import concourse.mybir as mybir
    import concourse.tile as tile
    from concourse.masks import make_identity

    f32 = mybir.dt.float32
    f32r = mybir.dt.float32r
    bf16 = mybir.dt.bfloat16
    i16 = mybir.dt.int16
    i32 = mybir.dt.int32
    GELU = (mybir.ActivationFunctionType.Tanh if sim_gelu
            else mybir.ActivationFunctionType.Gelu)

    nc = bacc.Bacc("TRN2", target_bir_lowering=False, debug=False,
                   enable_asserts=True, num_devices=N_CORES)

    # --- DRAM tensors (per core) ---
    t_obsidx = nc.dram_tensor("t_obsidx", [P, NI16], i16, kind="ExternalInput").ap()
    t_omask = nc.dram_tensor("t_omask", [P, NI16], i16, kind="ExternalInput").ap()
    t_attrf = nc.dram_tensor("t_attrf", [P, NJ], f32, kind="ExternalInput").ap()
    t_kmask = nc.dram_tensor("t_kmask", [N_ROWS, NF], f32, kind="ExternalInput").ap()
    t_obs = nc.dram_tensor("t_obs", [RPC, HID], bf16, kind="ExternalInput").ap()
    t_femb = nc.dram_tensor("t_femb", [NF, HID], f32, kind="ExternalInput").ap()
    t_w1 = nc.dram_tensor("t_w1", [NF, HID], f32, kind="ExternalInput").ap()
    t_b1 = nc.dram_tensor("t_b1", [HID], f32, kind="ExternalInput").ap()
    t_w2 = nc.dram_tensor("t_w2", [HID, NF], f32, kind="ExternalInput").ap()
    t_b2 = nc.dram_tensor("t_b2", [NF], f32, kind="ExternalInput").ap()
    t_wr = nc.dram_tensor("t_wr", [NF, HID], f32, kind="ExternalInput").ap()
    t_br = nc.dram_tensor("t_br", [HID], f32, kind="ExternalInput").ap()
    t_wc = nc.dram_tensor("t_wc", [HID, HID], f32, kind="ExternalInput").ap()
    t_bc = nc.dram_tensor("t_bc", [HID], f32, kind="ExternalInput").ap()
    t_out = nc.dram_tensor("t_out", [HID, ECP], f32, kind="ExternalOutput").ap()

    with tile.TileContext(nc) as tc:
        with tc.tile_pool(name="const", bufs=1) as cp, \
             tc.tile_pool(name="chunkp", bufs=2) as chp, \
             tc.tile_pool(name="work", bufs=3) as wp:

            # ---------- constants / weights ----------
            ident = cp.tile([P, P], f32)
            make_identity(nc, ident[:])
            identb = cp.tile([P, P], bf16)
            nc.vector.tensor_copy(out=identb[:], in_=ident[:])

            iota_i = cp.tile([P, NF], i32)
            nc.gpsimd.iota(iota_i[:], pattern=[[1, NF]], base=0, channel_multiplier=0)
            iota_f = cp.tile([P, NF], f32)
            nc.vector.tensor_copy(out=iota_f[:], in_=iota_i[:])

            # weights, cast to bf16 on chip (lower PE power, full stream rate)
            w1_sb = cp.tile([P, HID], bf16)         # rows 64:128 = rm_W1
            nc.gpsimd.dma_start(out=w1_sb[64:128, :], in_=t_w1[:])
            wstage = cp.tile([P, 2, HID], f32)
            nc.sync.dma_start(out=wstage[:, 0, 0:NF], in_=t_w2[0:128, :])
            nc.sync.dma_start(out=wstage[:, 1, 0:NF], in_=t_w2[128:256, :])
            w2_sb = cp.tile([P, 2, NF], bf16)       # [:,h,:] = rm_W2[128h:128h+128]
            nc.vector.tensor_copy(out=w2_sb[:], in_=wstage[:, :, 0:NF])
            wstage2 = cp.tile([NF, HID], f32)
            nc.sync.dma_start(out=wstage2[:], in_=t_wr[:])
            wr_sb = cp.tile([NF, HID], bf16)
            nc.vector.tensor_copy(out=wr_sb[:], in_=wstage2[:])
            wstage3 = cp.tile([P, 2, HID], f32)
            nc.sync.dma_start(out=wstage3[:, 0, :], in_=t_wc[0:128, :])
            nc.sync.dma_start(out=wstage3[:, 1, :], in_=t_wc[128:256, :])
            wc_sb = cp.tile([P, 2, HID], bf16)
            nc.vector.tensor_copy(out=wc_sb[:], in_=wstage3[:])

            # biases (per-partition columns)
            b1_sb = cp.tile([P, 2], f32)
            nc.sync.dma_start(out=b1_sb[:, 0:1], in_=t_b1[0:128, None])
            nc.sync.dma_start(out=b1_sb[:, 1:2], in_=t_b1[128:256, None])
            b2_sb = cp.tile([NF, 1], f32)
            nc.sync.dma_start(out=b2_sb[:], in_=t_b2[:, None])
            br_sb = cp.tile([P, 2], f32)
            nc.sync.dma_start(out=br_sb[:, 0:1], in_=t_br[0:128, None])
            nc.sync.dma_start(out=br_sb[:, 1:2], in_=t_br[128:256, None])
            bc_sb = cp.tile([P, 2], f32)
            nc.sync.dma_start(out=bc_sb[:, 0:1], in_=t_bc[0:128, None])
            nc.sync.dma_start(out=bc_sb[:, 1:2], in_=t_bc[128:256, None])

            # index arrays
            obsidx_sb = cp.tile([P, NI16], i16)
            nc.sync.dma_start(out=obsidx_sb[:], in_=t_obsidx[:])
            omask_sb = cp.tile([P, NI16], i16)
            nc.sync.dma_start(out=omask_sb[:], in_=t_omask[:])
            attr_sb = cp.tile([P, NJ], f32)
            nc.sync.dma_start(out=attr_sb[:], in_=t_attrf[:])

            # ---------- G = femb @ femb.T (bf16 for direct use with eqT) ---
            femb_sb = cp.tile([NF, HID], f32)
            nc.sync.dma_start(out=femb_sb[:], in_=t_femb[:])
            with tc.tile_pool(name="setup_ps", bufs=1, space="PSUM") as spp:
                ft_ps = spp.tile([P, 2, NF], f32, tag="setup")
                nc.tensor.transpose(out=ft_ps[:, 0, :], in_=femb_sb[:, 0:P],
                                    identity=ident[0:NF, 0:NF])
                nc.tensor.transpose(out=ft_ps[:, 1, :], in_=femb_sb[:, P:HID],
                                    identity=ident[0:NF, 0:NF])
                ft_sb = cp.tile([P, 2, NF], f32r)
                nc.vector.tensor_copy(out=ft_sb[:], in_=ft_ps[:])
                g_ps = spp.tile([NF, NF], f32, tag="setupg")
                nc.tensor.matmul(out=g_ps[:], lhsT=ft_sb[:, 0, :],
                                 rhs=ft_sb[:, 0, :], start=True, stop=False)
                nc.tensor.matmul(out=g_ps[:], lhsT=ft_sb[:, 1, :],
                                 rhs=ft_sb[:, 1, :], start=False, stop=True)
                g_sb = cp.tile([NF, NF], bf16)
                nc.vector.tensor_copy(out=g_sb[:], in_=g_ps[:])

            # ---------- main loop (1-tile software-pipeline skew) ----------
            _pp_cm = tc.tile_pool(name="psum", bufs=1, space="PSUM")
            pp = _pp_cm.__enter__()
            n_tiles = n_chunks * TILES_PER_CHUNK
            mi_bl = {}      # chunk -> [mi_t0, mi_t1]
            obsT_h = {}     # chunk -> obsT tile
            seqT_h = {}     # tile  -> seqT tile

            def emit_gathers(c):
                # known_mask gathers: two 1024-row blocks, each within one
                # mask-range region (sliced table keeps indices int16)
                bl = []
                for hb in range(2):
                    b = 2 * c + hb
                    q = BLK_REGION[b]
                    mi_t = chp.tile([P, MIBLK // P, NF], f32, tag=f"mi{hb}",
                                    name=f"mi_t{hb}", bufs=3)
                    nc.gpsimd.dma_gather(
                        out_ap=mi_t[:],
                        in_ap=t_kmask[MASK_BASES[q]:MASK_BASES[q + 1], :],
                        idxs_ap=omask_sb[:, b * (MIBLK // 16):
                                         (b + 1) * (MIBLK // 16)],
                        num_idxs=MIBLK, num_idxs_reg=MIBLK, elem_size=NF,
                        single_packet=False,
                    )
                    bl.append(mi_t)
                mi_bl[c] = bl
                # obs_h gather: 2048 rows of [256] bf16, row-major (the
                # transpose-gather's xbar writes starve DVE SBUF writes)
                obs_ch = chp.tile([P, CHUNK // P, HID], bf16, tag="obs",
                                  bufs=3)
                nc.gpsimd.dma_gather(
                    out_ap=obs_ch[:], in_ap=t_obs[:],
                    idxs_ap=obsidx_sb[:, c * (CHUNK // 16):
                                      (c + 1) * (CHUNK // 16)],
                    num_idxs=CHUNK, num_idxs_reg=CHUNK, elem_size=HID,
                    single_packet=False,
                )
                obsT_h[c] = obs_ch

            def emit_fe(t):
                # front end (row-major) + transpose to feat-major
                c, sti = t // TILES_PER_CHUNK, t % TILES_PER_CHUNK
                attr_v = attr_sb[:, t * NGROUP:(t + 1) * NGROUP]
                mi_v = mi_bl[c][sti // 2][:, (sti % 2) * NGROUP:
                                          (sti % 2 + 1) * NGROUP, :]

                stin = wp.tile([P, NGROUP, 2, NF], bf16, tag="stin")
                noteq = wp.tile([P, NGROUP, NF], f32, tag="noteq")
                nc.vector.tensor_tensor(
                    out=noteq[:],
                    in0=attr_v.unsqueeze(2).broadcast_to([P, NGROUP, NF]),
                    in1=iota_f[:].unsqueeze(1).broadcast_to([P, NGROUP, NF]),
                    op=mybir.AluOpType.not_equal,
                )
                # eq = 1 - noteq  -> transpose staging cols 0:64
                nc.vector.tensor_scalar(
                    out=stin[:, :, 0, :], in0=noteq[:],
                    scalar1=-1.0, scalar2=1.0,
                    op0=mybir.AluOpType.mult, op1=mybir.AluOpType.add,
                )
                m_sb = wp.tile([P, NGROUP, NF], f32, tag="m")
                nc.vector.tensor_tensor(
                    out=m_sb[:], in0=noteq[:], in1=mi_v,
                    op=mybir.AluOpType.mult,
                )
                n1 = wp.tile([P, NGROUP], f32, tag="n1")
                nc.vector.tensor_reduce(out=n1[:], in_=m_sb[:],
                                        axis=mybir.AxisListType.X,
                                        op=mybir.AluOpType.add)
                dden = wp.tile([P, NGROUP], f32, tag="dden")
                nc.vector.tensor_scalar(
                    out=dden[:], in0=n1[:], scalar1=EM1, scalar2=float(NF),
                    op0=mybir.AluOpType.mult, op1=mybir.AluOpType.add,
                )
                rr = wp.tile([P, NGROUP], f32, tag="rr")
                nc.vector.reciprocal_approx_fast(out=rr[:], in_=dden[:])
                st_sb = wp.tile([P, NGROUP, NF], f32, tag="st")
                nc.vector.tensor_scalar(
                    out=st_sb[:].rearrange("p g f -> p (g f)"),
                    in0=m_sb[:].rearrange("p g f -> p (g f)"),
                    scalar1=EM1, scalar2=1.0,
                    op0=mybir.AluOpType.mult, op1=mybir.AluOpType.add,
                )
                nc.vector.tensor_tensor(
                    out=stin[:, :, 1, :], in0=st_sb[:],
                    in1=rr[:].unsqueeze(2).broadcast_to([P, NGROUP, NF]),
                    op=mybir.AluOpType.mult,
                )

                stinT_ps = pp.tile([P, NGROUP, P], bf16, tag="xps", bufs=1)
                for g in range(NGROUP):
                    nc.tensor.transpose(
                        out=stinT_ps[:, g, :],
                        in_=stin[:, g, :, :].rearrange("p a f -> p (a f)"),
                        identity=identb[:])
                seqT = wp.tile([P, W], bf16, tag="seqT")
                nc.vector.tensor_copy(
                    out=seqT[:].rearrange("p (g e) -> p g e", g=NGROUP),
                    in_=stinT_ps[:])
                seqT_h[t] = seqT

            def emit_heavy(t):
                c, sti = t // TILES_PER_CHUNK, t % TILES_PER_CHUNK
                seqT = seqT_h.pop(t)
                eqT = seqT[0:NF, :]          # partitions 0:64 (bf16)
                sT = seqT[NF:P, :]           # partitions 64:128

                # aji^T = G @ eqT (bf16)
                aji_t = pp.tile([NF, W], f32, tag="aji")
                aji_ps = aji_t[:]
                nc.tensor.matmul(out=aji_ps, lhsT=g_sb[:],
                                 rhs=eqT, start=True, stop=True)

                # h1 = gelu(W1^T s + b1)   [256, W] in one 2-bank psum
                h1_ps = pp.tile([P, 2, W], f32, tag="h1h2")
                for h in range(2):
                    nc.tensor.matmul(out=h1_ps[:, h, :],
                                     lhsT=w1_sb[64:128, h * P:(h + 1) * P],
                                     rhs=sT, start=True, stop=True)
                h1T = wp.tile([P, 2, W], bf16, tag="h1T")
                for h in range(2):
                    nc.scalar.activation(out=h1T[:, h, :], in_=h1_ps[:, h, :],
                                         func=GELU, bias=b1_sb[:, h:h + 1],
                                         scale=1.0)

                # mJI = gelu(W2^T h1 + b2)  [64, W]
                mji_t = pp.tile([NF, W], f32, tag="mji")
                mji_ps = mji_t[:]
                nc.tensor.matmul(out=mji_ps, lhsT=w2_sb[:, 0, :],
                                 rhs=h1T[:, 0, :], start=True, stop=False)
                nc.tensor.matmul(out=mji_ps, lhsT=w2_sb[:, 1, :],
                                 rhs=h1T[:, 1, :], start=False, stop=True)
                mjiT = wp.tile([NF, W], f32, tag="mjiT")
                nc.scalar.activation(out=mjiT[:], in_=mji_ps,
                                     func=GELU, bias=b2_sb[:, 0:1], scale=1.0)

                # u = mJI * aji   [64, W]
                u_sb = wp.tile([NF, W], bf16, tag="u")
                nc.vector.tensor_tensor(out=u_sb[:], in0=mjiT[:],
                                        in1=aji_ps,
                                        op=mybir.AluOpType.mult)

                # h2 = gelu(Wr^T u + br)  [256, W]
                h2_ps = pp.tile([P, 2, W], f32, tag="h1h2")
                for h in range(2):
                    nc.tensor.matmul(out=h2_ps[:, h, :],
                                     lhsT=wr_sb[:, h * P:(h + 1) * P],
                                     rhs=u_sb[:], start=True, stop=True)
                h2T = wp.tile([P, 2, W], f32, tag="h2T")
                for h in range(2):
                    nc.scalar.activation(out=h2T[:, h, :], in_=h2_ps[:, h, :],
                                         func=GELU, bias=br_sb[:, h:h + 1],
                                         scale=1.0)

                # transpose obs_h rows to feat-major, then v = obs_h^T * h2
                obs_ch = obsT_h[c]
                oT_ps = pp.tile([P, 2, NGROUP, P], bf16, tag="oT")
                for h in range(2):
                    for g in range(NGROUP):
                        nc.tensor.transpose(
                            out=oT_ps[:, h, g, :],
                            in_=obs_ch[:, sti * NGROUP + g,
                                       h * P:(h + 1) * P],
                            identity=identb[:])
                vT = wp.tile([P, 2, W], bf16, tag="vT")
                nc.vector.tensor_tensor(
                    out=vT[:],
                    in0=oT_ps[:].rearrange("p h g e -> p h (g e)"),
                    in1=h2T[:], op=mybir.AluOpType.mult)

                # out^T = Wc^T v, feat-major [256, W]
                out_ps = pp.tile([P, 2, W], f32, tag="out")
                for mc in range(2):
                    for h in range(2):
                        nc.tensor.matmul(
                            out=out_ps[:, mc, :],
                            lhsT=wc_sb[:, h, mc * P:(mc + 1) * P],
                            rhs=vT[:, h, :],
                            start=(h == 0), stop=(h == 1))

                out_sb = wp.tile([P, 2, W], f32, tag="outsb")
                for mc in range(2):
                    nc.scalar.activation(
                        out=out_sb[:, mc, :], in_=out_ps[:, mc, :],
                        func=GELU, bias=bc_sb[:, mc:mc + 1], scale=1.0)

                dst = t_out[:, t * W:(t + 1) * W].rearrange(
                    "(c p) w -> p c w", p=P)
                nc.sync.dma_start(out=dst, in_=out_sb[:])

            emit_gathers(0)
            if n_chunks > 1:
                emit_gathers(1)
            emit_fe(0)
            for t in range(n_tiles):
                nt = t + 1
                if nt < n_tiles:
                    if nt % TILES_PER_CHUNK == 0:
                        nc2 = nt // TILES_PER_CHUNK + 1
                        if nc2 < n_chunks:
                            emit_gathers(nc2)
                    emit_fe(nt)
                emit_heavy(t)
            _pp_cm.__exit__(None, None, None)

    nc.compile()
    return nc


_NC_CACHE = {}


def _get_nc(sim_gelu=False, n_chunks=N_CHUNKS):
    key = (bool(sim_gelu), n_chunks)
    if key not in _NC_CACHE:
        _NC_CACHE[key] = build_nc(sim_gelu=key[0], n_chunks=key[1])
    return _NC_CACHE[key]


def _wrap16(a):
    """[ECP] int16 -> [128, ECP//16]: idx j at [j%16, j//16], replicated 8x
    across partition groups (one copy per Q7 core)."""
    w = np.ascontiguousarray(a.reshape(ECP // 16, 16).T)
    return np.ascontiguousarray(np.tile(w, (8, 1)))


def _wrapP(a, dtype):
    """[ECP] -> [128, ECP//128]: slot j*128+p at [p, j]."""
    return np.ascontiguousarray(a.astype(dtype).reshape(NJ, P).T)


def make_in_maps(known_mask, obs_idx, obs_mask_idx, attr_idx, obs_embs,
                 feature_emb, weights):
    """Bucket edges by (core = obs_idx // 12500, region = mask range), build
    per-core marshalled inputs. Returns (in_maps, slot_edge[8])."""
    f = np.float32
    obs_idx = np.asarray(obs_idx).ravel().astype(np.int64)
    obs_mask_idx = np.asarray(obs_mask_idx).ravel().astype(np.int64)
    attr_idx = np.asarray(attr_idx).ravel().astype(np.int64)

    known_mask = np.ascontiguousarray(known_mask, dtype=f)
    obs_embs_bf = np.ascontiguousarray(np.asarray(obs_embs, dtype=f).astype(BF16))
    feature_emb = np.ascontiguousarray(feature_emb, dtype=f)

    core_of = obs_idx // RPC
    region_of = np.searchsorted(MASK_BASES, obs_mask_idx, side="right") - 1

    in_maps = []
    slot_edge = []
    for k in range(N_CORES):
        loc_obs = np.zeros(ECP, np.int16)
        loc_msk = np.zeros(ECP, np.int16)
        loc_atr = np.zeros(ECP, f)
        s2e = np.full(ECP, -1, np.int64)
        base = 0
        for q in range(4):
            sel = np.nonzero((core_of == k) & (region_of == q))[0]
            n = sel.shape[0]
            if n > CAPQ[q]:
                raise RuntimeError(
                    f"bucket overflow core={k} region={q}: {n} > {CAPQ[q]}")
            sl = slice(base, base + n)
            loc_obs[sl] = (obs_idx[sel] - k * RPC).astype(np.int16)
            loc_msk[sl] = (obs_mask_idx[sel] - MASK_BASES[q]).astype(np.int16)
            loc_atr[sl] = attr_idx[sel].astype(f)
            s2e[sl] = sel
            base += CAPQ[q]
        in_maps.append({
            "t_obsidx": _wrap16(loc_obs),
            "t_omask": _wrap16(loc_msk),
            "t_attrf": _wrapP(loc_atr, f),
            "t_kmask": known_mask,
            "t_obs": np.ascontiguousarray(obs_embs_bf[k * RPC:(k + 1) * RPC]),
            "t_femb": feature_emb,
            **weights,
        })
        slot_edge.append(s2e)
    return in_maps, slot_edge


def kernel(known_mask, obs_idx, obs_mask_idx, attr_idx_need_to_be_impute,
           obs_embs, feature_emb,
           rm_W1, rm_b1, rm_W2, rm_b2, rr_W, rr_b, rc_W, rc_b,
           _sim_gelu=False, _trace=False):
    from concourse.bass_utils import run_bass_kernel_spmd

    f = np.float32
    weights = {
        "t_w1": np.ascontiguousarray(rm_W1, dtype=f),
        "t_b1": np.ascontiguousarray(rm_b1, dtype=f),
        "t_w2": np.ascontiguousarray(rm_W2, dtype=f),
        "t_b2": np.ascontiguousarray(rm_b2, dtype=f),
        "t_wr": np.ascontiguousarray(rr_W, dtype=f),
        "t_br": np.ascontiguousarray(rr_b, dtype=f),
        "t_wc": np.ascontiguousarray(rc_W, dtype=f),
        "t_bc": np.ascontiguousarray(rc_b, dtype=f),
    }
    in_maps, slot_edge = make_in_maps(
        known_mask, obs_idx, obs_mask_idx, attr_idx_need_to_be_impute,
        obs_embs, feature_emb, weights)

    nc = _get_nc(sim_gelu=_sim_gelu)
    res = run_bass_kernel_spmd(nc, in_maps, core_ids=list(range(N_CORES)),
                               trace=_trace)
    out = np.empty((E_TOT, HID), dtype=f)
    for k in range(N_CORES):
        s2e = slot_edge[k]
        valid = s2e >= 0
        out[s2e[valid]] = res.results[k]["t_out"][:, valid].T
    if _trace:
        kernel._last_results = res
    return out


# revision 35
# speedup vs baseline: 1.0797x; 1.0797x over previous
"""Trainium2 Bass kernel for nn_Attr_Relation_Net (gnn_message_passing).

Computation per edge e (E = 400000):
    obs_h  = obs_embs[obs_idx[e]]                 # [256] gather
    m_i    = known_mask[obs_mask_idx[e]]          # [64]  gather
    a      = attr_idx[e]
    a_j_i  = G[a]   where G = feature_emb @ feature_emb.T   (64x64, on device)
    m      = m_i with column a zeroed             # m_i * self_mask[a]
    s      = softmax(m) = (1 + (e-1)*m) / (64 + (e-1)*sum(m))   (m in {0,1})
    mJI    = gelu(gelu(s @ rm_W1 + rm_b1) @ rm_W2 + rm_b2)
    h2     = gelu((a_j_i * mJI) @ rr_W + rr_b)
    out[e] = gelu((obs_h * h2) @ rc_W + rc_b)

Sharding: edges are assigned to the 8 cores by obs_idx range (12500 rows of
obs_embs per core, so the obs_embs table is sharded by row and gather indices
fit int16 for the fast bulk-gather ucode). Within a core, edges are bucketed
into 4 obs_mask_idx ranges (<=32768 rows each) occupying fixed slot regions,
so known_mask gathers read a sliced table view with int16-local indices.
The host assembles per-core slot orderings and un-permutes the output.

On-chip layout: all matmul stages run feat-major ([feat, edges]). obs_embs is
stored bf16 in DRAM and bulk-gathered with transpose=True so obs_h arrives
feat-major directly (no PE transposes). The per-edge front end (masking +
closed-form softmax) runs row-major where per-edge broadcasts and reductions
are native; one PE transpose per 128 edges moves [eq | s] into feat-major.
a_j_i = G @ eq on PE (bf16). The final matmul is weight-stationary
(lhsT = rc_W chunks) producing feat-major output; rc_b is folded into the
GELU activation's per-partition bias; the output is written to DRAM as
[HID, ECP] and transposed back on the host. Matmuls run in float32r/bf16
(full PE rate at N>=256).
"""

import numpy as np

try:
    import ml_dtypes
    BF16 = np.dtype(ml_dtypes.bfloat16)
except ImportError:  # pragma: no cover
    BF16 = np.float32

E_TOT = 400000
N_CORES = 8
HID = 256
NF = 64
N_ROWS = 100000
P = 128

RPC = N_ROWS // N_CORES        # obs_embs rows per core (12500)
MASK_BASES = (0, 32768, 65536, 98304, 100000)
CAPQ = (17408, 17408, 17408, 1024)   # per-mask-range slot capacities
ECP = sum(CAPQ)                # 53248 slots per core
CHUNK = 2048                   # obs-gather granularity
MIBLK = 1024                   # known_mask-gather granularity
W = 512                        # compute tile (edges)
N_CHUNKS = ECP // CHUNK        # 26
TILES_PER_CHUNK = CHUNK // W   # 4
NGROUP = W // 128              # 4
NJ = ECP // P                  # 416 wrapped columns
NI16 = ECP // 16               # 3328 idx columns

EM1 = float(np.e - 1.0)

# region of each 1024-slot block
_CUM = np.cumsum((0,) + CAPQ)
BLK_REGION = [int(np.searchsorted(_CUM, b * MIBLK, side="right") - 1)
              for b in range(ECP // MIBLK)]


def build_nc(sim_gelu=False, n_chunks=N_CHUNKS):
    import concourse.bacc as bacc
    import concourse.mybir as mybir
    import concourse.tile as tile
    from concourse.masks import make_identity

    f32 = mybir.dt.float32
    f32r = mybir.dt.float32r
    bf16 = mybir.dt.bfloat16
    i16 = mybir.dt.int16
    i32 = mybir.dt.int32
    GELU = (mybir.ActivationFunctionType.Tanh if sim_gelu
            else mybir.ActivationFunctionType.Gelu)

    nc = bacc.Bacc("TRN2", target_bir_lowering=False, debug=False,
                   enable_asserts=True, num_devices=N_CORES)

    # --- DRAM tensors (per core) ---
    t_obsidx = nc.dram_tensor("t_obsidx", [P, NI16], i16, kind="ExternalInput").ap()
    t_omask = nc.dram_tensor("t_omask", [P, NI16], i16, kind="ExternalInput").ap()
    t_attrf = nc.dram_tensor("t_attrf", [P, NJ], f32, kind="ExternalInput").ap()
    t_kmask = nc.dram_tensor("t_kmask", [N_ROWS, NF], f32, kind="ExternalInput").ap()
    t_obs = nc.dram_tensor("t_obs", [RPC, HID], bf16, kind="ExternalInput").ap()
    t_femb = nc.dram_tensor("t_femb", [NF, HID], f32, kind="ExternalInput").ap()
    t_w1 = nc.dram_tensor("t_w1", [NF, HID], f32, kind="ExternalInput").ap()
    t_b1 = nc.dram_tensor("t_b1", [HID], f32, kind="ExternalInput").ap()
    t_w2 = nc.dram_tensor("t_w2", [HID, NF], f32, kind="ExternalInput").ap()
    t_b2 = nc.dram_tensor("t_b2", [NF], f32, kind="ExternalInput").ap()
    t_wr = nc.dram_tensor("t_wr", [NF, HID], f32, kind="ExternalInput").ap()
    t_br = nc.dram_tensor("t_br", [HID], f32, kind="ExternalInput").ap()
    t_wc = nc.dram_tensor("t_wc", [HID, HID], f32, kind="ExternalInput").ap()
    t_bc = nc.dram_tensor("t_bc", [HID], f32, kind="ExternalInput").ap()
    t_out = nc.dram_tensor("t_out", [HID, ECP], f32, kind="ExternalOutput").ap()

    with tile.TileContext(nc) as tc:
        with tc.tile_pool(name="const", bufs=1) as cp, \
             tc.tile_pool(name="chunkp", bufs=2) as chp, \
             tc.tile_pool(name="work", bufs=3) as wp:

            # ---------- constants / weights ----------
            ident = cp.tile([P, P], f32)
            make_identity(nc, ident[:])
            identb = cp.tile([P, P], bf16)
            nc.vector.tensor_copy(out=identb[:], in_=ident[:])

            iota_i = cp.tile([P, NF], i32)
            nc.gpsimd.iota(iota_i[:], pattern=[[1, NF]], base=0, channel_multiplier=0)
            iota_f = cp.tile([P, NF], f32)
            nc.vector.tensor_copy(out=iota_f[:], in_=iota_i[:])

            # weights, cast to bf16 on chip (lower PE power, full stream rate)
            w1_sb = cp.tile([P, HID], bf16)         # rows 64:128 = rm_W1
            nc.gpsimd.dma_start(out=w1_sb[64:128, :], in_=t_w1[:])
            wstage = cp.tile([P, 2, HID], f32)
            nc.sync.dma_start(out=wstage[:, 0, 0:NF], in_=t_w2[0:128, :])
            nc.sync.dma_start(out=wstage[:, 1, 0:NF], in_=t_w2[128:256, :])
            w2_sb = cp.tile([P, 2, NF], bf16)       # [:,h,:] = rm_W2[128h:128h+128]
            nc.vector.tensor_copy(out=w2_sb[:], in_=wstage[:, :, 0:NF])
            wstage2 = cp.tile([NF, HID], f32)
            nc.sync.dma_start(out=wstage2[:], in_=t_wr[:])
            wr_sb = cp.tile([NF, HID], bf16)
            nc.vector.tensor_copy(out=wr_sb[:], in_=wstage2[:])
            wstage3 = cp.tile([P, 2, HID], f32)
            nc.sync.dma_start(out=wstage3[:, 0, :], in_=t_wc[0:128, :])
            nc.sync.dma_start(out=wstage3[:, 1, :], in_=t_wc[128:256, :])
            wc_sb = cp.tile([P, 2, HID], bf16)
            nc.vector.tensor_copy(out=wc_sb[:], in_=wstage3[:])

            # biases (per-partition columns)
            b1_sb = cp.tile([P, 2], f32)
            nc.sync.dma_start(out=b1_sb[:, 0:1], in_=t_b1[0:128, None])
            nc.sync.dma_start(out=b1_sb[:, 1:2], in_=t_b1[128:256, None])
            b2_sb = cp.tile([NF, 1], f32)
            nc.sync.dma_start(out=b2_sb[:], in_=t_b2[:, None])
            br_sb = cp.tile([P, 2], f32)
            nc.sync.dma_start(out=br_sb[:, 0:1], in_=t_br[0:128, None])
            nc.sync.dma_start(out=br_sb[:, 1:2], in_=t_br[128:256, None])
            bc_sb = cp.tile([P, 2], f32)
            nc.sync.dma_start(out=bc_sb[:, 0:1], in_=t_bc[0:128, None])
            nc.sync.dma_start(out=bc_sb[:, 1:2], in_=t_bc[128:256, None])

            # index arrays
            obsidx_sb = cp.tile([P, NI16], i16)
            nc.sync.dma_start(out=obsidx_sb[:], in_=t_obsidx[:])
            omask_sb = cp.tile([P, NI16], i16)
            nc.sync.dma_start(out=omask_sb[:], in_=t_omask[:])
            attr_sb = cp.tile([P, NJ], f32)
            nc.sync.dma_start(out=attr_sb[:], in_=t_attrf[:])

            # ---------- G = femb @ femb.T (bf16 for direct use with eqT) ---
            femb_sb = cp.tile([NF, HID], f32)
            nc.sync.dma_start(out=femb_sb[:], in_=t_femb[:])
            with tc.tile_pool(name="setup_ps", bufs=1, space="PSUM") as spp:
                ft_ps = spp.tile([P, 2, NF], f32, tag="setup")
                nc.tensor.transpose(out=ft_ps[:, 0, :], in_=femb_sb[:, 0:P],
                                    identity=ident[0:NF, 0:NF])
                nc.tensor.transpose(out=ft_ps[:, 1, :], in_=femb_sb[:, P:HID],
                                    identity=ident[0:NF, 0:NF])
                ft_sb = cp.tile([P, 2, NF], f32r)
                nc.vector.tensor_copy(out=ft_sb[:], in_=ft_ps[:])
                g_ps = spp.tile([NF, NF], f32, tag="setupg")
                nc.tensor.matmul(out=g_ps[:], lhsT=ft_sb[:, 0, :],
                                 rhs=ft_sb[:, 0, :], start=True, stop=False)
                nc.tensor.matmul(out=g_ps[:], lhsT=ft_sb[:, 1, :],
                                 rhs=ft_sb[:, 1, :], start=False, stop=True)
                g_sb = cp.tile([NF, NF], bf16)
                nc.vector.tensor_copy(out=g_sb[:], in_=g_ps[:])

            # ---------- main loop (1-tile software-pipeline skew) ----------
            _pp_cm = tc.tile_pool(name="psum", bufs=1, space="PSUM")
            pp = _pp_cm.__enter__()
            n_tiles = n_chunks * TILES_PER_CHUNK
            mi_bl = {}      # chunk -> [mi_t0, mi_t1]
            obsT_h = {}     # chunk -> obsT tile
            seqT_h = {}     # tile  -> seqT tile

            def emit_gathers(c):
                # known_mask gathers: two 1024-row blocks, each within one
                # mask-range region (sliced table keeps indices int16)
                bl = []
                for hb in range(2):
                    b = 2 * c + hb
                    q = BLK_REGION[b]
                    mi_t = chp.tile([P, MIBLK // P, NF], f32, tag=f"mi{hb}",
                                    name=f"mi_t{hb}", bufs=3)
                    nc.gpsimd.dma_gather(
                        out_ap=mi_t[:],
                        in_ap=t_kmask[MASK_BASES[q]:MASK_BASES[q + 1], :],
                        idxs_ap=omask_sb[:, b * (MIBLK // 16):
                                         (b + 1) * (MIBLK // 16)],
                        num_idxs=MIBLK, num_idxs_reg=MIBLK, elem_size=NF,
                        single_packet=False,
                    )
                    bl.append(mi_t)
                mi_bl[c] = bl
                # obs_h gather: 2048 rows of [256] bf16, transposed on the
                # fly into feat-major [128, 2, 2048]
                obsT = chp.tile([P, 2, CHUNK], bf16, tag="obs", bufs=3)
                nc.gpsimd.dma_gather(
                    out_ap=obsT[:], in_ap=t_obs[:],
                    idxs_ap=obsidx_sb[:, c * (CHUNK // 16):
                                      (c + 1) * (CHUNK // 16)],
                    num_idxs=CHUNK, num_idxs_reg=CHUNK, elem_size=HID,
                    transpose=True, single_packet=False,
                )
                obsT_h[c] = obsT

            def emit_fe(t):
                # front end (row-major) + transpose to feat-major
                c, sti = t // TILES_PER_CHUNK, t % TILES_PER_CHUNK
                attr_v = attr_sb[:, t * NGROUP:(t + 1) * NGROUP]
                mi_v = mi_bl[c][sti // 2][:, (sti % 2) * NGROUP:
                                          (sti % 2 + 1) * NGROUP, :]

                stin = wp.tile([P, NGROUP, 2, NF], bf16, tag="stin")
                noteq = wp.tile([P, NGROUP, NF], f32, tag="noteq")
                nc.vector.tensor_tensor(
                    out=noteq[:],
                    in0=attr_v.unsqueeze(2).broadcast_to([P, NGROUP, NF]),
                    in1=iota_f[:].unsqueeze(1).broadcast_to([P, NGROUP, NF]),
                    op=mybir.AluOpType.not_equal,
                )
                # eq = 1 - noteq  -> transpose staging cols 0:64
                nc.vector.tensor_scalar(
                    out=stin[:, :, 0, :], in0=noteq[:],
                    scalar1=-1.0, scalar2=1.0,
                    op0=mybir.AluOpType.mult, op1=mybir.AluOpType.add,
                )
                m_sb = wp.tile([P, NGROUP, NF], f32, tag="m")
                nc.vector.tensor_tensor(
                    out=m_sb[:], in0=noteq[:], in1=mi_v,
                    op=mybir.AluOpType.mult,
                )
                n1 = wp.tile([P, NGROUP], f32, tag="n1")
                nc.vector.tensor_reduce(out=n1[:], in_=m_sb[:],
                                        axis=mybir.AxisListType.X,
                                        op=mybir.AluOpType.add)
                dden = wp.tile([P, NGROUP], f32, tag="dden")
                nc.vector.tensor_scalar(
                    out=dden[:], in0=n1[:], scalar1=EM1, scalar2=float(NF),
                    op0=mybir.AluOpType.mult, op1=mybir.AluOpType.add,
                )
                rr = wp.tile([P, NGROUP], f32, tag="rr")
                nc.vector.reciprocal_approx_fast(out=rr[:], in_=dden[:])
                st_sb = wp.tile([P, NGROUP, NF], f32, tag="st")
                nc.vector.tensor_scalar(
                    out=st_sb[:].rearrange("p g f -> p (g f)"),
                    in0=m_sb[:].rearrange("p g f -> p (g f)"),
                    scalar1=EM1, scalar2=1.0,
                    op0=mybir.AluOpType.mult, op1=mybir.AluOpType.add,
                )
                nc.vector.tensor_tensor(
                    out=stin[:, :, 1, :], in0=st_sb[:],
                    in1=rr[:].unsqueeze(2).broadcast_to([P, NGROUP, NF]),
                    op=mybir.AluOpType.mult,
                )

                stinT_ps = pp.tile([P, NGROUP, P], bf16, tag="xps", bufs=2)
                for g in range(NGROUP):
                    nc.tensor.transpose(
                        out=stinT_ps[:, g, :],
                        in_=stin[:, g, :, :].rearrange("p a f -> p (a f)"),
                        identity=identb[:])
                seqT = wp.tile([P, W], bf16, tag="seqT")
                nc.vector.tensor_copy(
                    out=seqT[:].rearrange("p (g e) -> p g e", g=NGROUP),
                    in_=stinT_ps[:])
                seqT_h[t] = seqT

            def emit_heavy(t):
                c, sti = t // TILES_PER_CHUNK, t % TILES_PER_CHUNK
                seqT = seqT_h.pop(t)
                eqT = seqT[0:NF, :]          # partitions 0:64 (bf16)
                sT = seqT[NF:P, :]           # partitions 64:128

                # aji^T = G @ eqT (bf16)
                aji_t = pp.tile([NF, W], f32, tag="aji")
                aji_ps = aji_t[:]
                nc.tensor.matmul(out=aji_ps, lhsT=g_sb[:],
                                 rhs=eqT, start=True, stop=True)

                # h1 = gelu(W1^T s + b1)   [256, W] in one 2-bank psum
                h1_ps = pp.tile([P, 2, W], f32, tag="h1h2")
                for h in range(2):
                    nc.tensor.matmul(out=h1_ps[:, h, :],
                                     lhsT=w1_sb[64:128, h * P:(h + 1) * P],
                                     rhs=sT, start=True, stop=True)
                h1T = wp.tile([P, 2, W], bf16, tag="h1T")
                for h in range(2):
                    nc.scalar.activation(out=h1T[:, h, :], in_=h1_ps[:, h, :],
                                         func=GELU, bias=b1_sb[:, h:h + 1],
                                         scale=1.0)

                # mJI = gelu(W2^T h1 + b2)  [64, W]
                mji_t = pp.tile([NF, W], f32, tag="mji")
                mji_ps = mji_t[:]
                nc.tensor.matmul(out=mji_ps, lhsT=w2_sb[:, 0, :],
                                 rhs=h1T[:, 0, :], start=True, stop=False)
                nc.tensor.matmul(out=mji_ps, lhsT=w2_sb[:, 1, :],
                                 rhs=h1T[:, 1, :], start=False, stop=True)
                mjiT = wp.tile([NF, W], f32, tag="mjiT")
                nc.scalar.activation(out=mjiT[:], in_=mji_ps,
                                     func=GELU, bias=b2_sb[:, 0:1], scale=1.0)

                # u = mJI * aji   [64, W]
                u_sb = wp.tile([NF, W], bf16, tag="u")
                nc.vector.tensor_tensor(out=u_sb[:], in0=mjiT[:],
                                        in1=aji_ps,
                                        op=mybir.AluOpType.mult)

                # h2 = gelu(Wr^T u + br)  [256, W]
                h2_ps = pp.tile([P, 2, W], f32, tag="h1h2")
                for h in range(2):
                    nc.tensor.matmul(out=h2_ps[:, h, :],
                                     lhsT=wr_sb[:, h * P:(h + 1) * P],
                                     rhs=u_sb[:], start=True, stop=True)
                h2T = wp.tile([P, 2, W], f32, tag="h2T")
                for h in range(2):
                    nc.scalar.activation(out=h2T[:, h, :], in_=h2_ps[:, h, :],
                                         func=GELU, bias=br_sb[:, h:h + 1],
                                         scale=1.0)

                # v = obs_h^T * h2 (obs_h already feat-major)
                vT = wp.tile([P, 2, W], bf16, tag="vT")
                nc.vector.tensor_tensor(
                    out=vT[:],
                    in0=obsT_h[c][:, :, sti * W:(sti + 1) * W],
                    in1=h2T[:], op=mybir.AluOpType.mult)

                # out^T = Wc^T v, feat-major [256, W]
                out_ps = pp.tile([P, 2, W], f32, tag="out")
                for mc in range(2):
                    for h in range(2):
                        nc.tensor.matmul(
                            out=out_ps[:, mc, :],
                            lhsT=wc_sb[:, h, mc * P:(mc + 1) * P],
                            rhs=vT[:, h, :],
                            start=(h == 0), stop=(h == 1))

                out_sb = wp.tile([P, 2, W], f32, tag="outsb")
                for mc in range(2):
                    nc.scalar.activation(
                        out=out_sb[:, mc, :], in_=out_ps[:, mc, :],
                        func=GELU, bias=bc_sb[:, mc:mc + 1], scale=1.0)

                dst = t_out[:, t * W:(t + 1) * W].rearrange(
                    "(c p) w -> p c w", p=P)
                nc.sync.dma_start(out=dst, in_=out_sb[:])

            # FE runs 2 tiles ahead but is emitted AFTER heavy(t): the
            # stall-prone FE vector ops (gather/DVE SBUF port contention)
            # sit behind the latency-critical u/vT ops in the DVE queue,
            # and the 2-tile skew gives slack to absorb their stalls.
            emit_gathers(0)
            if n_chunks > 1:
                emit_gathers(1)
            emit_fe(0)
            if n_tiles > 1:
                emit_fe(1)
            for t in range(n_tiles):
                emit_heavy(t)
                nt = t + 2
                if nt < n_tiles:
                    if nt % TILES_PER_CHUNK == 0:
                        nc2 = nt // TILES_PER_CHUNK + 1
                        if nc2 < n_chunks:
                            emit_gathers(nc2)
                    emit_fe(nt)
            _pp_cm.__exit__(None, None, None)

    nc.compile()
    return nc


_NC_CACHE = {}


def _get_nc(sim_gelu=False, n_chunks=N_CHUNKS):
    key = (bool(sim_gelu), n_chunks)
    if key not in _NC_CACHE:
        _NC_CACHE[key] = build_nc(sim_gelu=key[0], n_chunks=key[1])
    return _NC_CACHE[key]


def _wrap16(a):
    """[ECP] int16 -> [128, ECP//16]: idx j at [j%16, j//16], replicated 8x
    across partition groups (one copy per Q7 core)."""
    w = np.ascontiguousarray(a.reshape(ECP // 16, 16).T)
    return np.ascontiguousarray(np.tile(w, (8, 1)))


def _wrapP(a, dtype):
    """[ECP] -> [128, ECP//128]: slot j*128+p at [p, j]."""
    return np.ascontiguousarray(a.astype(dtype).reshape(NJ, P).T)


def make_in_maps(known_mask, obs_idx, obs_mask_idx, attr_idx, obs_embs,
                 feature_emb, weights):
    """Bucket edges by (core = obs_idx // 12500, region = mask range), build
    per-core marshalled inputs. Returns (in_maps, slot_edge[8])."""
    f = np.float32
    obs_idx = np.asarray(obs_idx).ravel().astype(np.int64)
    obs_mask_idx = np.asarray(obs_mask_idx).ravel().astype(np.int64)
    attr_idx = np.asarray(attr_idx).ravel().astype(np.int64)

    known_mask = np.ascontiguousarray(known_mask, dtype=f)
    obs_embs_bf = np.ascontiguousarray(np.asarray(obs_embs, dtype=f).astype(BF16))
    feature_emb = np.ascontiguousarray(feature_emb, dtype=f)

    core_of = obs_idx // RPC
    region_of = np.searchsorted(MASK_BASES, obs_mask_idx, side="right") - 1

    in_maps = []
    slot_edge = []
    for k in range(N_CORES):
        loc_obs = np.zeros(ECP, np.int16)
        loc_msk = np.zeros(ECP, np.int16)
        loc_atr = np.zeros(ECP, f)
        s2e = np.full(ECP, -1, np.int64)
        base = 0
        for q in range(4):
            sel = np.nonzero((core_of == k) & (region_of == q))[0]
            n = sel.shape[0]
            if n > CAPQ[q]:
                raise RuntimeError(
                    f"bucket overflow core={k} region={q}: {n} > {CAPQ[q]}")
            sl = slice(base, base + n)
            loc_obs[sl] = (obs_idx[sel] - k * RPC).astype(np.int16)
            loc_msk[sl] = (obs_mask_idx[sel] - MASK_BASES[q]).astype(np.int16)
            loc_atr[sl] = attr_idx[sel].astype(f)
            s2e[sl] = sel
            base += CAPQ[q]
        in_maps.append({
            "t_obsidx": _wrap16(loc_obs),
            "t_omask": _wrap16(loc_msk),
            "t_attrf": _wrapP(loc_atr, f),
            "t_kmask": known_mask,
            "t_obs": np.ascontiguousarray(obs_embs_bf[k * RPC:(k + 1) * RPC]),
            "t_femb": feature_emb,
            **weights,
        })
        slot_edge.append(s2e)
    return in_maps, slot_edge


def kernel(known_mask, obs_idx, obs_mask_idx, attr_idx_need_to_be_impute,
           obs_embs, feature_emb,
           rm_W1, rm_b1, rm_W2, rm_b2, rr_W, rr_b, rc_W, rc_b,
           _sim_gelu=False, _trace=False):
    from concourse.bass_utils import run_bass_kernel_spmd

    f = np.float32
    weights = {
        "t_w1": np.ascontiguousarray(rm_W1, dtype=f),
        "t_b1": np.ascontiguousarray(rm_b1, dtype=f),
        "t_w2": np.ascontiguousarray(rm_W2, dtype=f),
        "t_b2": np.ascontiguousarray(rm_b2, dtype=f),
        "t_wr": np.ascontiguousarray(rr_W, dtype=f),
        "t_br": np.ascontiguousarray(rr_b, dtype=f),
        "t_wc": np.ascontiguousarray(rc_W, dtype=f),
        "t_bc": np.ascontiguousarray(rc_b, dtype=f),
    }
    in_maps, slot_edge = make_in_maps(
        known_mask, obs_idx, obs_mask_idx, attr_idx_need_to_be_impute,
        obs_embs, feature_emb, weights)

    nc = _get_nc(sim_gelu=_sim_gelu)
    res = run_bass_kernel_spmd(nc, in_maps, core_ids=list(range(N_CORES)),
                               trace=_trace)
    out = np.empty((E_TOT, HID), dtype=f)
    for k in range(N_CORES):
        s2e = slot_edge[k]
        valid = s2e >= 0
        out[s2e[valid]] = res.results[k]["t_out"][:, valid].T
    if _trace:
        kernel._last_results = res
    return out


# revision 38
# speedup vs baseline: 1.0967x; 1.0157x over previous
"""Trainium2 Bass kernel for nn_Attr_Relation_Net (gnn_message_passing).

Computation per edge e (E = 400000):
    obs_h  = obs_embs[obs_idx[e]]                 # [256] gather
    m_i    = known_mask[obs_mask_idx[e]]          # [64]  gather
    a      = attr_idx[e]
    a_j_i  = G[a]   where G = feature_emb @ feature_emb.T   (64x64, on device)
    m      = m_i with column a zeroed             # m_i * self_mask[a]
    s      = softmax(m) = (1 + (e-1)*m) / (64 + (e-1)*sum(m))   (m in {0,1})
    mJI    = gelu(gelu(s @ rm_W1 + rm_b1) @ rm_W2 + rm_b2)
    h2     = gelu((a_j_i * mJI) @ rr_W + rr_b)
    out[e] = gelu((obs_h * h2) @ rc_W + rc_b)

Sharding: edges are assigned to the 8 cores by obs_idx range (12500 rows of
obs_embs per core, so the obs_embs table is sharded by row and gather indices
fit int16 for the fast bulk-gather ucode). Within a core, edges are bucketed
into 4 obs_mask_idx ranges (<=32768 rows each) occupying fixed slot regions,
so known_mask gathers read a sliced table view with int16-local indices.
The host assembles per-core slot orderings and un-permutes the output.

On-chip layout: all matmul stages run feat-major ([feat, edges]). obs_embs is
stored bf16 in DRAM and bulk-gathered with transpose=True so obs_h arrives
feat-major directly (no PE transposes). The per-edge front end (masking +
closed-form softmax) runs row-major where per-edge broadcasts and reductions
are native; one PE transpose per 128 edges moves [eq | s] into feat-major.
a_j_i = G @ eq on PE (bf16). The final matmul is weight-stationary
(lhsT = rc_W chunks) producing feat-major output; rc_b is folded into the
GELU activation's per-partition bias; the output is written to DRAM as
[HID, ECP] and transposed back on the host. All matmuls run in bf16 with
f32 PSUM accumulation (full PE stream rate, lowest PE power under the
board's 4-of-8 duty-cycle throttle). The emission order software-pipelines
tiles: front end of tile t+1 is emitted before the heavy phase of tile t.
"""

import numpy as np

try:
    import ml_dtypes
    BF16 = np.dtype(ml_dtypes.bfloat16)
except ImportError:  # pragma: no cover
    BF16 = np.float32

E_TOT = 400000
N_CORES = 8
HID = 256
NF = 64
N_ROWS = 100000
P = 128

RPC = N_ROWS // N_CORES        # obs_embs rows per core (12500)
MASK_BASES = (0, 32768, 65536, 98304, 100000)
CAPQ = (17408, 17408, 17408, 1024)   # per-mask-range slot capacities
ECP = sum(CAPQ)                # 53248 slots per core
CHUNK = 2048                   # obs-gather granularity
MIBLK = 1024                   # known_mask-gather granularity
W = 512                        # compute tile (edges)
N_CHUNKS = ECP // CHUNK        # 26
TILES_PER_CHUNK = CHUNK // W   # 4
NGROUP = W // 128              # 4
NJ = ECP // P                  # 416 wrapped columns
NI16 = ECP // 16               # 3328 idx columns

EM1 = float(np.e - 1.0)

# region of each 1024-slot block
_CUM = np.cumsum((0,) + CAPQ)
BLK_REGION = [int(np.searchsorted(_CUM, b * MIBLK, side="right") - 1)
              for b in range(ECP // MIBLK)]


def build_nc(sim_gelu=False, n_chunks=N_CHUNKS):
    import concourse.bacc as bacc
    import concourse.mybir as mybir
    import concourse.tile as tile
    from concourse.masks import make_identity

    f32 = mybir.dt.float32
    f32r = mybir.dt.float32r
    bf16 = mybir.dt.bfloat16
    i16 = mybir.dt.int16
    i32 = mybir.dt.int32
    GELU = (mybir.ActivationFunctionType.Tanh if sim_gelu
            else mybir.ActivationFunctionType.Gelu)

    nc = bacc.Bacc("TRN2", target_bir_lowering=False, debug=False,
                   enable_asserts=True, num_devices=N_CORES)

    # --- DRAM tensors (per core) ---
    t_obsidx = nc.dram_tensor("t_obsidx", [P, NI16], i16, kind="ExternalInput").ap()
    t_omask = nc.dram_tensor("t_omask", [P, NI16], i16, kind="ExternalInput").ap()
    t_attrf = nc.dram_tensor("t_attrf", [P, NJ], f32, kind="ExternalInput").ap()
    t_kmask = nc.dram_tensor("t_kmask", [N_ROWS, NF], f32, kind="ExternalInput").ap()
    t_obs = nc.dram_tensor("t_obs", [RPC, HID], bf16, kind="ExternalInput").ap()
    t_femb = nc.dram_tensor("t_femb", [NF, HID], f32, kind="ExternalInput").ap()
    t_w1 = nc.dram_tensor("t_w1", [NF, HID], f32, kind="ExternalInput").ap()
    t_b1 = nc.dram_tensor("t_b1", [HID], f32, kind="ExternalInput").ap()
    t_w2 = nc.dram_tensor("t_w2", [HID, NF], f32, kind="ExternalInput").ap()
    t_b2 = nc.dram_tensor("t_b2", [NF], f32, kind="ExternalInput").ap()
    t_wr = nc.dram_tensor("t_wr", [NF, HID], f32, kind="ExternalInput").ap()
    t_br = nc.dram_tensor("t_br", [HID], f32, kind="ExternalInput").ap()
    t_wc = nc.dram_tensor("t_wc", [HID, HID], f32, kind="ExternalInput").ap()
    t_bc = nc.dram_tensor("t_bc", [HID], f32, kind="ExternalInput").ap()
    t_out = nc.dram_tensor("t_out", [HID, ECP], f32, kind="ExternalOutput").ap()

    with tile.TileContext(nc) as tc:
        with tc.tile_pool(name="const", bufs=1) as cp, \
             tc.tile_pool(name="chunkp", bufs=2) as chp, \
             tc.tile_pool(name="work", bufs=2) as wp:

            # ---------- constants / weights ----------
            ident = cp.tile([P, P], f32)
            make_identity(nc, ident[:])
            identb = cp.tile([P, P], bf16)
            nc.vector.tensor_copy(out=identb[:], in_=ident[:])

            iota_i = cp.tile([P, NF], i32)
            nc.gpsimd.iota(iota_i[:], pattern=[[1, NF]], base=0, channel_multiplier=0)
            iota_f = cp.tile([P, NF], f32)
            nc.vector.tensor_copy(out=iota_f[:], in_=iota_i[:])

            # weights, cast to bf16 on chip (lower PE power, full stream rate)
            w1_sb = cp.tile([P, HID], bf16)         # rows 64:128 = rm_W1
            nc.gpsimd.dma_start(out=w1_sb[64:128, :], in_=t_w1[:])
            wstage = cp.tile([P, 2, HID], f32)
            nc.sync.dma_start(out=wstage[:, 0, 0:NF], in_=t_w2[0:128, :])
            nc.sync.dma_start(out=wstage[:, 1, 0:NF], in_=t_w2[128:256, :])
            w2_sb = cp.tile([P, 2, NF], bf16)       # [:,h,:] = rm_W2[128h:128h+128]
            nc.vector.tensor_copy(out=w2_sb[:], in_=wstage[:, :, 0:NF])
            wstage2 = cp.tile([NF, HID], f32)
            nc.sync.dma_start(out=wstage2[:], in_=t_wr[:])
            wr_sb = cp.tile([NF, HID], bf16)
            nc.vector.tensor_copy(out=wr_sb[:], in_=wstage2[:])
            wstage3 = cp.tile([P, 2, HID], f32)
            nc.sync.dma_start(out=wstage3[:, 0, :], in_=t_wc[0:128, :])
            nc.sync.dma_start(out=wstage3[:, 1, :], in_=t_wc[128:256, :])
            wc_sb = cp.tile([P, 2, HID], bf16)
            nc.vector.tensor_copy(out=wc_sb[:], in_=wstage3[:])

            # biases (per-partition columns)
            b1_sb = cp.tile([P, 2], f32)
            nc.sync.dma_start(out=b1_sb[:, 0:1], in_=t_b1[0:128, None])
            nc.sync.dma_start(out=b1_sb[:, 1:2], in_=t_b1[128:256, None])
            b2_sb = cp.tile([NF, 1], f32)
            nc.sync.dma_start(out=b2_sb[:], in_=t_b2[:, None])
            br_sb = cp.tile([P, 2], f32)
            nc.sync.dma_start(out=br_sb[:, 0:1], in_=t_br[0:128, None])
            nc.sync.dma_start(out=br_sb[:, 1:2], in_=t_br[128:256, None])
            bc_sb = cp.tile([P, 2], f32)
            nc.sync.dma_start(out=bc_sb[:, 0:1], in_=t_bc[0:128, None])
            nc.sync.dma_start(out=bc_sb[:, 1:2], in_=t_bc[128:256, None])

            # index arrays
            obsidx_sb = cp.tile([P, NI16], i16)
            nc.sync.dma_start(out=obsidx_sb[:], in_=t_obsidx[:])
            omask_sb = cp.tile([P, NI16], i16)
            nc.sync.dma_start(out=omask_sb[:], in_=t_omask[:])
            attr_sb = cp.tile([P, NJ], f32)
            nc.sync.dma_start(out=attr_sb[:], in_=t_attrf[:])

            # ---------- G = femb @ femb.T (bf16 for direct use with eqT) ---
            femb_sb = cp.tile([NF, HID], f32)
            nc.sync.dma_start(out=femb_sb[:], in_=t_femb[:])
            with tc.tile_pool(name="setup_ps", bufs=1, space="PSUM") as spp:
                ft_ps = spp.tile([P, 2, NF], f32, tag="setup")
                nc.tensor.transpose(out=ft_ps[:, 0, :], in_=femb_sb[:, 0:P],
                                    identity=ident[0:NF, 0:NF])
                nc.tensor.transpose(out=ft_ps[:, 1, :], in_=femb_sb[:, P:HID],
                                    identity=ident[0:NF, 0:NF])
                ft_sb = cp.tile([P, 2, NF], f32r)
                nc.vector.tensor_copy(out=ft_sb[:], in_=ft_ps[:])
                g_ps = spp.tile([NF, NF], f32, tag="setupg")
                nc.tensor.matmul(out=g_ps[:], lhsT=ft_sb[:, 0, :],
                                 rhs=ft_sb[:, 0, :], start=True, stop=False)
                nc.tensor.matmul(out=g_ps[:], lhsT=ft_sb[:, 1, :],
                                 rhs=ft_sb[:, 1, :], start=False, stop=True)
                g_sb = cp.tile([NF, NF], bf16)
                nc.vector.tensor_copy(out=g_sb[:], in_=g_ps[:])

            # ---------- main loop (1-tile software-pipeline skew) ----------
            _pp_cm = tc.tile_pool(name="psum", bufs=1, space="PSUM")
            pp = _pp_cm.__enter__()
            n_tiles = n_chunks * TILES_PER_CHUNK
            mi_bl = {}      # chunk -> [mi_t0, mi_t1]
            obsT_h = {}     # chunk -> obsT tile
            seqT_h = {}     # tile  -> seqT tile

            def emit_gathers(c):
                # known_mask gathers: two 1024-row blocks, each within one
                # mask-range region (sliced table keeps indices int16)
                bl = []
                for hb in range(2):
                    b = 2 * c + hb
                    q = BLK_REGION[b]
                    mi_t = chp.tile([P, MIBLK // P, NF], f32, tag=f"mi{hb}",
                                    name=f"mi_t{hb}", bufs=3)
                    nc.gpsimd.dma_gather(
                        out_ap=mi_t[:],
                        in_ap=t_kmask[MASK_BASES[q]:MASK_BASES[q + 1], :],
                        idxs_ap=omask_sb[:, b * (MIBLK // 16):
                                         (b + 1) * (MIBLK // 16)],
                        num_idxs=MIBLK, num_idxs_reg=MIBLK, elem_size=NF,
                        single_packet=False,
                    )
                    bl.append(mi_t)
                mi_bl[c] = bl
                # obs_h gather: 2048 rows of [256] bf16, transposed on the
                # fly into feat-major [128, 2, 2048]
                obsT = chp.tile([P, 2, CHUNK], bf16, tag="obs", bufs=3)
                nc.gpsimd.dma_gather(
                    out_ap=obsT[:], in_ap=t_obs[:],
                    idxs_ap=obsidx_sb[:, c * (CHUNK // 16):
                                      (c + 1) * (CHUNK // 16)],
                    num_idxs=CHUNK, num_idxs_reg=CHUNK, elem_size=HID,
                    transpose=True, single_packet=False,
                )
                obsT_h[c] = obsT

            def emit_fe(t):
                # front end (row-major) + transpose to feat-major
                c, sti = t // TILES_PER_CHUNK, t % TILES_PER_CHUNK
                attr_v = attr_sb[:, t * NGROUP:(t + 1) * NGROUP]
                mi_v = mi_bl[c][sti // 2][:, (sti % 2) * NGROUP:
                                          (sti % 2 + 1) * NGROUP, :]

                stin = wp.tile([P, NGROUP, 2, NF], bf16, tag="stin")
                noteq = wp.tile([P, NGROUP, NF], f32, tag="noteq")
                nc.vector.tensor_tensor(
                    out=noteq[:],
                    in0=attr_v.unsqueeze(2).broadcast_to([P, NGROUP, NF]),
                    in1=iota_f[:].unsqueeze(1).broadcast_to([P, NGROUP, NF]),
                    op=mybir.AluOpType.not_equal,
                )
                # eq = 1 - noteq  -> transpose staging cols 0:64
                nc.vector.tensor_scalar(
                    out=stin[:, :, 0, :], in0=noteq[:],
                    scalar1=-1.0, scalar2=1.0,
                    op0=mybir.AluOpType.mult, op1=mybir.AluOpType.add,
                )
                m_sb = wp.tile([P, NGROUP, NF], f32, tag="m")
                nc.vector.tensor_tensor(
                    out=m_sb[:], in0=noteq[:], in1=mi_v,
                    op=mybir.AluOpType.mult,
                )
                n1 = wp.tile([P, NGROUP], f32, tag="n1")
                nc.vector.tensor_reduce(out=n1[:], in_=m_sb[:],
                                        axis=mybir.AxisListType.X,
                                        op=mybir.AluOpType.add)
                dden = wp.tile([P, NGROUP], f32, tag="dden")
                nc.vector.tensor_scalar(
                    out=dden[:], in0=n1[:], scalar1=EM1, scalar2=float(NF),
                    op0=mybir.AluOpType.mult, op1=mybir.AluOpType.add,
                )
                rr = wp.tile([P, NGROUP], f32, tag="rr")
                nc.vector.reciprocal_approx_fast(out=rr[:], in_=dden[:])
                st_sb = wp.tile([P, NGROUP, NF], f32, tag="st")
                nc.vector.tensor_scalar(
                    out=st_sb[:].rearrange("p g f -> p (g f)"),
                    in0=m_sb[:].rearrange("p g f -> p (g f)"),
                    scalar1=EM1, scalar2=1.0,
                    op0=mybir.AluOpType.mult, op1=mybir.AluOpType.add,
                )
                nc.vector.tensor_tensor(
                    out=stin[:, :, 1, :], in0=st_sb[:],
                    in1=rr[:].unsqueeze(2).broadcast_to([P, NGROUP, NF]),
                    op=mybir.AluOpType.mult,
                )

                stinT_ps = pp.tile([P, NGROUP, P], bf16, tag="xps", bufs=2)
                for g in range(NGROUP):
                    nc.tensor.transpose(
                        out=stinT_ps[:, g, :],
                        in_=stin[:, g, :, :].rearrange("p a f -> p (a f)"),
                        identity=identb[:])
                seqT = wp.tile([P, W], bf16, tag="seqT")
                nc.vector.tensor_copy(
                    out=seqT[:].rearrange("p (g e) -> p g e", g=NGROUP),
                    in_=stinT_ps[:])
                seqT_h[t] = seqT

            def emit_heavy(t):
                c, sti = t // TILES_PER_CHUNK, t % TILES_PER_CHUNK
                seqT = seqT_h.pop(t)
                eqT = seqT[0:NF, :]          # partitions 0:64 (bf16)
                sT = seqT[NF:P, :]           # partitions 64:128

                # aji^T = G @ eqT (bf16)
                aji_t = pp.tile([NF, W], f32, tag="aji")
                aji_ps = aji_t[:]
                nc.tensor.matmul(out=aji_ps, lhsT=g_sb[:],
                                 rhs=eqT, start=True, stop=True)

                # h1 = gelu(W1^T s + b1)   [256, W] in one 2-bank psum
                h1_ps = pp.tile([P, 2, W], f32, tag="h1h2")
                for h in range(2):
                    nc.tensor.matmul(out=h1_ps[:, h, :],
                                     lhsT=w1_sb[64:128, h * P:(h + 1) * P],
                                     rhs=sT, start=True, stop=True)
                h1T = wp.tile([P, 2, W], bf16, tag="h1T")
                for h in range(2):
                    nc.scalar.activation(out=h1T[:, h, :], in_=h1_ps[:, h, :],
                                         func=GELU, bias=b1_sb[:, h:h + 1],
                                         scale=1.0)

                # mJI = gelu(W2^T h1 + b2)  [64, W]
                mji_t = pp.tile([NF, W], f32, tag="mji")
                mji_ps = mji_t[:]
                nc.tensor.matmul(out=mji_ps, lhsT=w2_sb[:, 0, :],
                                 rhs=h1T[:, 0, :], start=True, stop=False)
                nc.tensor.matmul(out=mji_ps, lhsT=w2_sb[:, 1, :],
                                 rhs=h1T[:, 1, :], start=False, stop=True)
                mjiT = wp.tile([NF, W], f32, tag="mjiT")
                nc.scalar.activation(out=mjiT[:], in_=mji_ps,
                                     func=GELU, bias=b2_sb[:, 0:1], scale=1.0)

                # u = mJI * aji   [64, W]
                u_sb = wp.tile([NF, W], bf16, tag="u")
                nc.vector.tensor_tensor(out=u_sb[:], in0=mjiT[:],
                                        in1=aji_ps,
                                        op=mybir.AluOpType.mult)

                # h2 = gelu(Wr^T u + br)  [256, W]
                h2_ps = pp.tile([P, 2, W], f32, tag="h1h2")
                for h in range(2):
                    nc.tensor.matmul(out=h2_ps[:, h, :],
                                     lhsT=wr_sb[:, h * P:(h + 1) * P],
                                     rhs=u_sb[:], start=True, stop=True)
                h2T = wp.tile([P, 2, W], f32, tag="h2T")
                for h in range(2):
                    nc.scalar.activation(out=h2T[:, h, :], in_=h2_ps[:, h, :],
                                         func=GELU, bias=br_sb[:, h:h + 1],
                                         scale=1.0)

                # v = obs_h^T * h2 (obs_h already feat-major)
                vT = wp.tile([P, 2, W], bf16, tag="vT")
                nc.vector.tensor_tensor(
                    out=vT[:],
                    in0=obsT_h[c][:, :, sti * W:(sti + 1) * W],
                    in1=h2T[:], op=mybir.AluOpType.mult)

                # out^T = Wc^T v, feat-major [256, W]
                out_ps = pp.tile([P, 2, W], f32, tag="out")
                for mc in range(2):
                    for h in range(2):
                        nc.tensor.matmul(
                            out=out_ps[:, mc, :],
                            lhsT=wc_sb[:, h, mc * P:(mc + 1) * P],
                            rhs=vT[:, h, :],
                            start=(h == 0), stop=(h == 1))

                out_sb = wp.tile([P, 2, W], f32, tag="outsb")
                for mc in range(2):
                    nc.scalar.activation(
                        out=out_sb[:, mc, :], in_=out_ps[:, mc, :],
                        func=GELU, bias=bc_sb[:, mc:mc + 1], scale=1.0)

                dst = t_out[:, t * W:(t + 1) * W].rearrange(
                    "(c p) w -> p c w", p=P)
                nc.sync.dma_start(out=dst, in_=out_sb[:])

            # 1-tile software-pipeline skew: tile t+1's front end is
            # emitted before tile t's heavy phase.
            emit_gathers(0)
            emit_fe(0)
            for t in range(n_tiles):
                nt = t + 1
                if nt < n_tiles:
                    if nt % TILES_PER_CHUNK == 0:
                        emit_gathers(nt // TILES_PER_CHUNK)
                    emit_fe(nt)
                emit_heavy(t)
            _pp_cm.__exit__(None, None, None)

    nc.compile()
    return nc


_NC_CACHE = {}


def _get_nc(sim_gelu=False, n_chunks=N_CHUNKS):
    key = (bool(sim_gelu), n_chunks)
    if key not in _NC_CACHE:
        _NC_CACHE[key] = build_nc(sim_gelu=key[0], n_chunks=key[1])
    return _NC_CACHE[key]


def _wrap16(a):
    """[ECP] int16 -> [128, ECP//16]: idx j at [j%16, j//16], replicated 8x
    across partition groups (one copy per Q7 core)."""
    w = np.ascontiguousarray(a.reshape(ECP // 16, 16).T)
    return np.ascontiguousarray(np.tile(w, (8, 1)))


def _wrapP(a, dtype):
    """[ECP] -> [128, ECP//128]: slot j*128+p at [p, j]."""
    return np.ascontiguousarray(a.astype(dtype).reshape(NJ, P).T)


def make_in_maps(known_mask, obs_idx, obs_mask_idx, attr_idx, obs_embs,
                 feature_emb, weights):
    """Bucket edges by (core = obs_idx // 12500, region = mask range), build
    per-core marshalled inputs. Returns (in_maps, slot_edge[8])."""
    f = np.float32
    obs_idx = np.asarray(obs_idx).ravel().astype(np.int64)
    obs_mask_idx = np.asarray(obs_mask_idx).ravel().astype(np.int64)
    attr_idx = np.asarray(attr_idx).ravel().astype(np.int64)

    known_mask = np.ascontiguousarray(known_mask, dtype=f)
    obs_embs_bf = np.ascontiguousarray(np.asarray(obs_embs, dtype=f).astype(BF16))
    feature_emb = np.ascontiguousarray(feature_emb, dtype=f)

    core_of = obs_idx // RPC
    region_of = np.searchsorted(MASK_BASES, obs_mask_idx, side="right") - 1

    in_maps = []
    slot_edge = []
    for k in range(N_CORES):
        loc_obs = np.zeros(ECP, np.int16)
        loc_msk = np.zeros(ECP, np.int16)
        loc_atr = np.zeros(ECP, f)
        s2e = np.full(ECP, -1, np.int64)
        base = 0
        for q in range(4):
            sel = np.nonzero((core_of == k) & (region_of == q))[0]
            n = sel.shape[0]
            if n > CAPQ[q]:
                raise RuntimeError(
                    f"bucket overflow core={k} region={q}: {n} > {CAPQ[q]}")
            sl = slice(base, base + n)
            loc_obs[sl] = (obs_idx[sel] - k * RPC).astype(np.int16)
            loc_msk[sl] = (obs_mask_idx[sel] - MASK_BASES[q]).astype(np.int16)
            loc_atr[sl] = attr_idx[sel].astype(f)
            s2e[sl] = sel
            base += CAPQ[q]
        in_maps.append({
            "t_obsidx": _wrap16(loc_obs),
            "t_omask": _wrap16(loc_msk),
            "t_attrf": _wrapP(loc_atr, f),
            "t_kmask": known_mask,
            "t_obs": np.ascontiguousarray(obs_embs_bf[k * RPC:(k + 1) * RPC]),
            "t_femb": feature_emb,
            **weights,
        })
        slot_edge.append(s2e)
    return in_maps, slot_edge


def kernel(known_mask, obs_idx, obs_mask_idx, attr_idx_need_to_be_impute,
           obs_embs, feature_emb,
           rm_W1, rm_b1, rm_W2, rm_b2, rr_W, rr_b, rc_W, rc_b,
           _sim_gelu=False, _trace=False):
    from concourse.bass_utils import run_bass_kernel_spmd

    f = np.float32
    weights = {
        "t_w1": np.ascontiguousarray(rm_W1, dtype=f),
        "t_b1": np.ascontiguousarray(rm_b1, dtype=f),
        "t_w2": np.ascontiguousarray(rm_W2, dtype=f),
        "t_b2": np.ascontiguousarray(rm_b2, dtype=f),
        "t_wr": np.ascontiguousarray(rr_W, dtype=f),
        "t_br": np.ascontiguousarray(rr_b, dtype=f),
        "t_wc": np.ascontiguousarray(rc_W, dtype=f),
        "t_bc": np.ascontiguousarray(rc_b, dtype=f),
    }
    in_maps, slot_edge = make_in_maps(
        known_mask, obs_idx, obs_mask_idx, attr_idx_need_to_be_impute,
        obs_embs, feature_emb, weights)

    nc = _get_nc(sim_gelu=_sim_gelu)
    res = run_bass_kernel_spmd(nc, in_maps, core_ids=list(range(N_CORES)),
                               trace=_trace)
    out = np.empty((E_TOT, HID), dtype=f)
    for k in range(N_CORES):
        s2e = slot_edge[k]
        valid = s2e >= 0
        out[s2e[valid]] = res.results[k]["t_out"][:, valid].T
    if _trace:
        kernel._last_results = res
    return out


# revision 42
# speedup vs baseline: 1.1797x; 1.0757x over previous
"""Trainium2 Bass kernel for nn_Attr_Relation_Net (gnn_message_passing).

Computation per edge e (E = 400000):
    obs_h  = obs_embs[obs_idx[e]]                 # [256] gather
    m_i    = known_mask[obs_mask_idx[e]]          # [64]  gather
    a      = attr_idx[e]
    a_j_i  = G[a]   where G = feature_emb @ feature_emb.T   (64x64, on device)
    m      = m_i with column a zeroed             # m_i * self_mask[a]
    s      = softmax(m) = (1 + (e-1)*m) / (64 + (e-1)*sum(m))   (m in {0,1})
    mJI    = gelu(gelu(s @ rm_W1 + rm_b1) @ rm_W2 + rm_b2)
    h2     = gelu((a_j_i * mJI) @ rr_W + rr_b)
    out[e] = gelu((obs_h * h2) @ rc_W + rc_b)

Sharding: edges are assigned to the 8 cores by obs_idx range (12500 rows of
obs_embs per core, so the obs_embs table is sharded by row and gather indices
fit int16 for the fast bulk-gather ucode). Within a core, edges are bucketed
into 4 obs_mask_idx ranges (<=32768 rows each) occupying fixed slot regions,
so known_mask gathers read a sliced table view with int16-local indices.
The host assembles per-core slot orderings and un-permutes the output.

On-chip layout: all matmul stages run feat-major ([feat, edges]). obs_embs is
stored bf16 in DRAM and bulk-gathered with transpose=True so obs_h arrives
feat-major directly (no PE transposes). The per-edge front end (masking +
closed-form softmax) runs row-major where per-edge broadcasts and reductions
are native; one PE transpose per 128 edges moves [eq | s] into feat-major.
a_j_i = G @ eq on PE (bf16). The final matmul is weight-stationary
(lhsT = rc_W chunks) producing feat-major output; rc_b is folded into the
GELU activation's per-partition bias; the output is written to DRAM as
[HID, ECP] and transposed back on the host. All matmuls run in bf16 with
f32 PSUM accumulation (full PE stream rate, lowest PE power under the
board's 4-of-8 duty-cycle throttle). The emission order software-pipelines
tiles: front end of tile t+1 is emitted before the heavy phase of tile t.
"""

import numpy as np

try:
    import ml_dtypes
    BF16 = np.dtype(ml_dtypes.bfloat16)
except ImportError:  # pragma: no cover
    BF16 = np.float32

E_TOT = 400000
N_CORES = 8
HID = 256
NF = 64
N_ROWS = 100000
P = 128

RPC = N_ROWS // N_CORES        # obs_embs rows per core (12500)
MASK_BASES = (0, 32768, 65536, 98304, 100000)
CAPQ = (17408, 17408, 17408, 1024)   # per-mask-range slot capacities
ECP = sum(CAPQ)                # 53248 slots per core
CHUNK = 2048                   # obs-gather granularity
MIBLK = 1024                   # known_mask-gather granularity
W = 512                        # compute tile (edges)
N_CHUNKS = ECP // CHUNK        # 26
TILES_PER_CHUNK = CHUNK // W   # 4
NGROUP = W // 128              # 4
NJ = ECP // P                  # 416 wrapped columns
NI16 = ECP // 16               # 3328 idx columns

EM1 = float(np.e - 1.0)

# region of each 1024-slot block
_CUM = np.cumsum((0,) + CAPQ)
BLK_REGION = [int(np.searchsorted(_CUM, b * MIBLK, side="right") - 1)
              for b in range(ECP // MIBLK)]


def build_nc(sim_gelu=False, n_chunks=N_CHUNKS):
    import concourse.bacc as bacc
    import concourse.mybir as mybir
    import concourse.tile as tile
    from concourse.masks import make_identity

    f32 = mybir.dt.float32
    f32r = mybir.dt.float32r
    bf16 = mybir.dt.bfloat16
    i16 = mybir.dt.int16
    i32 = mybir.dt.int32
    GELU = (mybir.ActivationFunctionType.Tanh if sim_gelu
            else mybir.ActivationFunctionType.Gelu)

    nc = bacc.Bacc("TRN2", target_bir_lowering=False, debug=False,
                   enable_asserts=True, num_devices=N_CORES)

    # --- DRAM tensors (per core) ---
    t_obsidx = nc.dram_tensor("t_obsidx", [P, NI16], i16, kind="ExternalInput").ap()
    t_omask = nc.dram_tensor("t_omask", [P, NI16], i16, kind="ExternalInput").ap()
    t_attrf = nc.dram_tensor("t_attrf", [P, NJ], f32, kind="ExternalInput").ap()
    t_kmask = nc.dram_tensor("t_kmask", [N_ROWS, NF], f32, kind="ExternalInput").ap()
    t_obs = nc.dram_tensor("t_obs", [RPC, HID], bf16, kind="ExternalInput").ap()
    t_femb = nc.dram_tensor("t_femb", [NF, HID], f32, kind="ExternalInput").ap()
    t_w1 = nc.dram_tensor("t_w1", [NF, HID], f32, kind="ExternalInput").ap()
    t_b1 = nc.dram_tensor("t_b1", [HID], f32, kind="ExternalInput").ap()
    t_w2 = nc.dram_tensor("t_w2", [HID, NF], f32, kind="ExternalInput").ap()
    t_b2 = nc.dram_tensor("t_b2", [NF], f32, kind="ExternalInput").ap()
    t_wr = nc.dram_tensor("t_wr", [NF, HID], f32, kind="ExternalInput").ap()
    t_br = nc.dram_tensor("t_br", [HID], f32, kind="ExternalInput").ap()
    t_wc = nc.dram_tensor("t_wc", [HID, HID], f32, kind="ExternalInput").ap()
    t_bc = nc.dram_tensor("t_bc", [HID], f32, kind="ExternalInput").ap()
    t_out = nc.dram_tensor("t_out", [HID, ECP], f32, kind="ExternalOutput").ap()

    with tile.TileContext(nc) as tc:
        with tc.tile_pool(name="const", bufs=1) as cp, \
             tc.tile_pool(name="chunkp", bufs=2) as chp, \
             tc.tile_pool(name="work", bufs=2) as wp:

            # ---------- constants / weights ----------
            ident = cp.tile([P, P], f32)
            make_identity(nc, ident[:])
            identb = cp.tile([P, P], bf16)
            nc.vector.tensor_copy(out=identb[:], in_=ident[:])

            iota_i = cp.tile([P, NF], i32)
            nc.gpsimd.iota(iota_i[:], pattern=[[1, NF]], base=0, channel_multiplier=0)
            iota_f = cp.tile([P, NF], f32)
            nc.vector.tensor_copy(out=iota_f[:], in_=iota_i[:])

            # weights, cast to bf16 on chip (lower PE power, full stream rate)
            w1_sb = cp.tile([P, HID], bf16)         # rows 64:128 = rm_W1
            nc.gpsimd.dma_start(out=w1_sb[64:128, :], in_=t_w1[:])
            wstage = cp.tile([P, 2, HID], f32)
            nc.sync.dma_start(out=wstage[:, 0, 0:NF], in_=t_w2[0:128, :])
            nc.sync.dma_start(out=wstage[:, 1, 0:NF], in_=t_w2[128:256, :])
            w2_sb = cp.tile([P, 2, NF], bf16)       # [:,h,:] = rm_W2[128h:128h+128]
            nc.vector.tensor_copy(out=w2_sb[:], in_=wstage[:, :, 0:NF])
            wstage2 = cp.tile([NF, HID], f32)
            nc.sync.dma_start(out=wstage2[:], in_=t_wr[:])
            wr_sb = cp.tile([NF, HID], bf16)
            nc.vector.tensor_copy(out=wr_sb[:], in_=wstage2[:])
            wstage3 = cp.tile([P, 2, HID], f32)
            nc.sync.dma_start(out=wstage3[:, 0, :], in_=t_wc[0:128, :])
            nc.sync.dma_start(out=wstage3[:, 1, :], in_=t_wc[128:256, :])
            wc_sb = cp.tile([P, 2, HID], bf16)
            nc.vector.tensor_copy(out=wc_sb[:], in_=wstage3[:])

            # biases (per-partition columns)
            b1_sb = cp.tile([P, 2], f32)
            nc.sync.dma_start(out=b1_sb[:, 0:1], in_=t_b1[0:128, None])
            nc.sync.dma_start(out=b1_sb[:, 1:2], in_=t_b1[128:256, None])
            b2_sb = cp.tile([NF, 1], f32)
            nc.sync.dma_start(out=b2_sb[:], in_=t_b2[:, None])
            br_sb = cp.tile([P, 2], f32)
            nc.sync.dma_start(out=br_sb[:, 0:1], in_=t_br[0:128, None])
            nc.sync.dma_start(out=br_sb[:, 1:2], in_=t_br[128:256, None])
            bc_sb = cp.tile([P, 2], f32)
            nc.sync.dma_start(out=bc_sb[:, 0:1], in_=t_bc[0:128, None])
            nc.sync.dma_start(out=bc_sb[:, 1:2], in_=t_bc[128:256, None])

            # index arrays
            obsidx_sb = cp.tile([P, NI16], i16)
            nc.sync.dma_start(out=obsidx_sb[:], in_=t_obsidx[:])
            omask_sb = cp.tile([P, NI16], i16)
            nc.sync.dma_start(out=omask_sb[:], in_=t_omask[:])
            attr_sb = cp.tile([P, NJ], f32)
            nc.sync.dma_start(out=attr_sb[:], in_=t_attrf[:])

            # ---------- G = femb @ femb.T (bf16 for direct use with eqT) ---
            femb_sb = cp.tile([NF, HID], f32)
            nc.sync.dma_start(out=femb_sb[:], in_=t_femb[:])
            with tc.tile_pool(name="setup_ps", bufs=1, space="PSUM") as spp:
                ft_ps = spp.tile([P, 2, NF], f32, tag="setup")
                nc.tensor.transpose(out=ft_ps[:, 0, :], in_=femb_sb[:, 0:P],
                                    identity=ident[0:NF, 0:NF])
                nc.tensor.transpose(out=ft_ps[:, 1, :], in_=femb_sb[:, P:HID],
                                    identity=ident[0:NF, 0:NF])
                ft_sb = cp.tile([P, 2, NF], f32r)
                nc.vector.tensor_copy(out=ft_sb[:], in_=ft_ps[:])
                g_ps = spp.tile([NF, NF], f32, tag="setupg")
                nc.tensor.matmul(out=g_ps[:], lhsT=ft_sb[:, 0, :],
                                 rhs=ft_sb[:, 0, :], start=True, stop=False)
                nc.tensor.matmul(out=g_ps[:], lhsT=ft_sb[:, 1, :],
                                 rhs=ft_sb[:, 1, :], start=False, stop=True)
                g_sb = cp.tile([NF, NF], bf16)
                nc.vector.tensor_copy(out=g_sb[:], in_=g_ps[:])

            # ---------- main loop (1-tile software-pipeline skew) ----------
            _pp_cm = tc.tile_pool(name="psum", bufs=1, space="PSUM")
            pp = _pp_cm.__enter__()
            n_tiles = n_chunks * TILES_PER_CHUNK
            mi_bl = {}      # chunk -> [mi_t0, mi_t1]
            obsT_h = {}     # chunk -> obsT tile
            seqT_h = {}     # tile  -> seqT tile

            def emit_gathers(c):
                # known_mask gathers: two 1024-row blocks, each within one
                # mask-range region (sliced table keeps indices int16)
                bl = []
                for hb in range(2):
                    b = 2 * c + hb
                    q = BLK_REGION[b]
                    mi_t = chp.tile([P, MIBLK // P, NF], f32, tag=f"mi{hb}",
                                    name=f"mi_t{hb}", bufs=3)
                    nc.gpsimd.dma_gather(
                        out_ap=mi_t[:],
                        in_ap=t_kmask[MASK_BASES[q]:MASK_BASES[q + 1], :],
                        idxs_ap=omask_sb[:, b * (MIBLK // 16):
                                         (b + 1) * (MIBLK // 16)],
                        num_idxs=MIBLK, num_idxs_reg=MIBLK, elem_size=NF,
                        single_packet=False,
                    )
                    bl.append(mi_t)
                mi_bl[c] = bl
                # obs_h gather: 2048 rows of [256] bf16, transposed on the
                # fly into feat-major [128, 2, 2048]
                obsT = chp.tile([P, 2, CHUNK], bf16, tag="obs", bufs=3)
                nc.gpsimd.dma_gather(
                    out_ap=obsT[:], in_ap=t_obs[:],
                    idxs_ap=obsidx_sb[:, c * (CHUNK // 16):
                                      (c + 1) * (CHUNK // 16)],
                    num_idxs=CHUNK, num_idxs_reg=CHUNK, elem_size=HID,
                    transpose=True, single_packet=False,
                )
                obsT_h[c] = obsT

            def emit_fe(t):
                # front end (row-major) + transpose to feat-major
                c, sti = t // TILES_PER_CHUNK, t % TILES_PER_CHUNK
                attr_v = attr_sb[:, t * NGROUP:(t + 1) * NGROUP]
                mi_v = mi_bl[c][sti // 2][:, (sti % 2) * NGROUP:
                                          (sti % 2 + 1) * NGROUP, :]

                stin = wp.tile([P, NGROUP, 2, NF], bf16, tag="stin")
                noteq = wp.tile([P, NGROUP, NF], f32, tag="noteq")
                nc.vector.tensor_tensor(
                    out=noteq[:],
                    in0=attr_v.unsqueeze(2).broadcast_to([P, NGROUP, NF]),
                    in1=iota_f[:].unsqueeze(1).broadcast_to([P, NGROUP, NF]),
                    op=mybir.AluOpType.not_equal,
                )
                # eq = 1 - noteq  -> transpose staging cols 0:64.
                # On the scalar engine: its SBUF port is not starved by
                # concurrent gathers, unlike DVE writes.
                nc.scalar.activation(
                    out=stin[:, :, 0, :], in_=noteq[:],
                    func=mybir.ActivationFunctionType.Copy,
                    bias=1.0, scale=-1.0,
                )
                m_sb = wp.tile([P, NGROUP, NF], f32, tag="m")
                nc.vector.tensor_tensor(
                    out=m_sb[:], in0=noteq[:], in1=mi_v,
                    op=mybir.AluOpType.mult,
                )
                n1 = wp.tile([P, NGROUP], f32, tag="n1")
                nc.vector.tensor_reduce(out=n1[:], in_=m_sb[:],
                                        axis=mybir.AxisListType.X,
                                        op=mybir.AluOpType.add)
                dden = wp.tile([P, NGROUP], f32, tag="dden")
                nc.vector.tensor_scalar(
                    out=dden[:], in0=n1[:], scalar1=EM1, scalar2=float(NF),
                    op0=mybir.AluOpType.mult, op1=mybir.AluOpType.add,
                )
                rr = wp.tile([P, NGROUP], f32, tag="rr")
                nc.vector.reciprocal_approx_fast(out=rr[:], in_=dden[:])
                rr2 = wp.tile([P, NGROUP], f32, tag="rr2")
                nc.vector.tensor_scalar(
                    out=rr2[:], in0=rr[:], scalar1=EM1, scalar2=None,
                    op0=mybir.AluOpType.mult,
                )
                # s = (1 + EM1*m)*rr = m*(EM1*rr) + rr: per-group scalar
                # activations with per-partition scale/bias APs (keeps the
                # stall-prone strided writes off the DVE)
                for g in range(NGROUP):
                    nc.scalar.activation(
                        out=stin[:, g, 1, :], in_=m_sb[:, g, :],
                        func=mybir.ActivationFunctionType.Identity,
                        bias=rr[:, g:g + 1], scale=rr2[:, g:g + 1],
                    )

                stinT_ps = pp.tile([P, NGROUP, P], bf16, tag="xps", bufs=2)
                for g in range(NGROUP):
                    nc.tensor.transpose(
                        out=stinT_ps[:, g, :],
                        in_=stin[:, g, :, :].rearrange("p a f -> p (a f)"),
                        identity=identb[:])
                seqT = wp.tile([P, W], bf16, tag="seqT")
                nc.scalar.activation(
                    out=seqT[:].rearrange("p (g e) -> p g e", g=NGROUP),
                    in_=stinT_ps[:],
                    func=mybir.ActivationFunctionType.Copy, scale=1.0)
                seqT_h[t] = seqT

            def emit_heavy(t):
                c, sti = t // TILES_PER_CHUNK, t % TILES_PER_CHUNK
                seqT = seqT_h.pop(t)
                eqT = seqT[0:NF, :]          # partitions 0:64 (bf16)
                sT = seqT[NF:P, :]           # partitions 64:128

                # aji^T = G @ eqT (bf16)
                aji_t = pp.tile([NF, W], f32, tag="aji")
                aji_ps = aji_t[:]
                nc.tensor.matmul(out=aji_ps, lhsT=g_sb[:],
                                 rhs=eqT, start=True, stop=True)

                # h1 = gelu(W1^T s + b1)   [256, W] in one 2-bank psum
                h1_ps = pp.tile([P, 2, W], f32, tag="h1h2")
                for h in range(2):
                    nc.tensor.matmul(out=h1_ps[:, h, :],
                                     lhsT=w1_sb[64:128, h * P:(h + 1) * P],
                                     rhs=sT, start=True, stop=True)
                h1T = wp.tile([P, 2, W], bf16, tag="h1T")
                for h in range(2):
                    nc.scalar.activation(out=h1T[:, h, :], in_=h1_ps[:, h, :],
                                         func=GELU, bias=b1_sb[:, h:h + 1],
                                         scale=1.0)

                # mJI = gelu(W2^T h1 + b2)  [64, W]
                mji_t = pp.tile([NF, W], f32, tag="mji")
                mji_ps = mji_t[:]
                nc.tensor.matmul(out=mji_ps, lhsT=w2_sb[:, 0, :],
                                 rhs=h1T[:, 0, :], start=True, stop=False)
                nc.tensor.matmul(out=mji_ps, lhsT=w2_sb[:, 1, :],
                                 rhs=h1T[:, 1, :], start=False, stop=True)
                mjiT = wp.tile([NF, W], f32, tag="mjiT")
                nc.scalar.activation(out=mjiT[:], in_=mji_ps,
                                     func=GELU, bias=b2_sb[:, 0:1], scale=1.0)

                # u = mJI * aji   [64, W]
                u_sb = wp.tile([NF, W], bf16, tag="u")
                nc.vector.tensor_tensor(out=u_sb[:], in0=mjiT[:],
                                        in1=aji_ps,
                                        op=mybir.AluOpType.mult)

                # h2 = gelu(Wr^T u + br)  [256, W]
                h2_ps = pp.tile([P, 2, W], f32, tag="h1h2")
                for h in range(2):
                    nc.tensor.matmul(out=h2_ps[:, h, :],
                                     lhsT=wr_sb[:, h * P:(h + 1) * P],
                                     rhs=u_sb[:], start=True, stop=True)
                h2T = wp.tile([P, 2, W], f32, tag="h2T")
                for h in range(2):
                    nc.scalar.activation(out=h2T[:, h, :], in_=h2_ps[:, h, :],
                                         func=GELU, bias=br_sb[:, h:h + 1],
                                         scale=1.0)

                # v = obs_h^T * h2 (obs_h already feat-major)
                vT = wp.tile([P, 2, W], bf16, tag="vT")
                nc.vector.tensor_tensor(
                    out=vT[:],
                    in0=obsT_h[c][:, :, sti * W:(sti + 1) * W],
                    in1=h2T[:], op=mybir.AluOpType.mult)

                # out^T = Wc^T v, feat-major [256, W]
                out_ps = pp.tile([P, 2, W], f32, tag="out")
                for mc in range(2):
                    for h in range(2):
                        nc.tensor.matmul(
                            out=out_ps[:, mc, :],
                            lhsT=wc_sb[:, h, mc * P:(mc + 1) * P],
                            rhs=vT[:, h, :],
                            start=(h == 0), stop=(h == 1))

                out_sb = wp.tile([P, 2, W], f32, tag="outsb")
                for mc in range(2):
                    nc.scalar.activation(
                        out=out_sb[:, mc, :], in_=out_ps[:, mc, :],
                        func=GELU, bias=bc_sb[:, mc:mc + 1], scale=1.0)

                dst = t_out[:, t * W:(t + 1) * W].rearrange(
                    "(c p) w -> p c w", p=P)
                nc.sync.dma_start(out=dst, in_=out_sb[:])

            # 1-tile software-pipeline skew: tile t+1's front end is
            # emitted before tile t's heavy phase.
            emit_gathers(0)
            emit_fe(0)
            for t in range(n_tiles):
                nt = t + 1
                if nt < n_tiles:
                    if nt % TILES_PER_CHUNK == 0:
                        emit_gathers(nt // TILES_PER_CHUNK)
                    emit_fe(nt)
                emit_heavy(t)
            _pp_cm.__exit__(None, None, None)

    nc.compile()
    return nc


_NC_CACHE = {}


def _get_nc(sim_gelu=False, n_chunks=N_CHUNKS):
    key = (bool(sim_gelu), n_chunks)
    if key not in _NC_CACHE:
        _NC_CACHE[key] = build_nc(sim_gelu=key[0], n_chunks=key[1])
    return _NC_CACHE[key]


def _wrap16(a):
    """[ECP] int16 -> [128, ECP//16]: idx j at [j%16, j//16], replicated 8x
    across partition groups (one copy per Q7 core)."""
    w = np.ascontiguousarray(a.reshape(ECP // 16, 16).T)
    return np.ascontiguousarray(np.tile(w, (8, 1)))


def _wrapP(a, dtype):
    """[ECP] -> [128, ECP//128]: slot j*128+p at [p, j]."""
    return np.ascontiguousarray(a.astype(dtype).reshape(NJ, P).T)


def make_in_maps(known_mask, obs_idx, obs_mask_idx, attr_idx, obs_embs,
                 feature_emb, weights):
    """Bucket edges by (core = obs_idx // 12500, region = mask range), build
    per-core marshalled inputs. Returns (in_maps, slot_edge[8])."""
    f = np.float32
    obs_idx = np.asarray(obs_idx).ravel().astype(np.int64)
    obs_mask_idx = np.asarray(obs_mask_idx).ravel().astype(np.int64)
    attr_idx = np.asarray(attr_idx).ravel().astype(np.int64)

    known_mask = np.ascontiguousarray(known_mask, dtype=f)
    obs_embs_bf = np.ascontiguousarray(np.asarray(obs_embs, dtype=f).astype(BF16))
    feature_emb = np.ascontiguousarray(feature_emb, dtype=f)

    core_of = obs_idx // RPC
    region_of = np.searchsorted(MASK_BASES, obs_mask_idx, side="right") - 1

    in_maps = []
    slot_edge = []
    for k in range(N_CORES):
        loc_obs = np.zeros(ECP, np.int16)
        loc_msk = np.zeros(ECP, np.int16)
        loc_atr = np.zeros(ECP, f)
        s2e = np.full(ECP, -1, np.int64)
        base = 0
        for q in range(4):
            sel = np.nonzero((core_of == k) & (region_of == q))[0]
            n = sel.shape[0]
            if n > CAPQ[q]:
                raise RuntimeError(
                    f"bucket overflow core={k} region={q}: {n} > {CAPQ[q]}")
            sl = slice(base, base + n)
            loc_obs[sl] = (obs_idx[sel] - k * RPC).astype(np.int16)
            loc_msk[sl] = (obs_mask_idx[sel] - MASK_BASES[q]).astype(np.int16)
            loc_atr[sl] = attr_idx[sel].astype(f)
            s2e[sl] = sel
            base += CAPQ[q]
        in_maps.append({
            "t_obsidx": _wrap16(loc_obs),
            "t_omask": _wrap16(loc_msk),
            "t_attrf": _wrapP(loc_atr, f),
            "t_kmask": known_mask,
            "t_obs": np.ascontiguousarray(obs_embs_bf[k * RPC:(k + 1) * RPC]),
            "t_femb": feature_emb,
            **weights,
        })
        slot_edge.append(s2e)
    return in_maps, slot_edge


def kernel(known_mask, obs_idx, obs_mask_idx, attr_idx_need_to_be_impute,
           obs_embs, feature_emb,
           rm_W1, rm_b1, rm_W2, rm_b2, rr_W, rr_b, rc_W, rc_b,
           _sim_gelu=False, _trace=False):
    from concourse.bass_utils import run_bass_kernel_spmd

    f = np.float32
    weights = {
        "t_w1": np.ascontiguousarray(rm_W1, dtype=f),
        "t_b1": np.ascontiguousarray(rm_b1, dtype=f),
        "t_w2": np.ascontiguousarray(rm_W2, dtype=f),
        "t_b2": np.ascontiguousarray(rm_b2, dtype=f),
        "t_wr": np.ascontiguousarray(rr_W, dtype=f),
        "t_br": np.ascontiguousarray(rr_b, dtype=f),
        "t_wc": np.ascontiguousarray(rc_W, dtype=f),
        "t_bc": np.ascontiguousarray(rc_b, dtype=f),
    }
    in_maps, slot_edge = make_in_maps(
        known_mask, obs_idx, obs_mask_idx, attr_idx_need_to_be_impute,
        obs_embs, feature_emb, weights)

    nc = _get_nc(sim_gelu=_sim_gelu)
    res = run_bass_kernel_spmd(nc, in_maps, core_ids=list(range(N_CORES)),
                               trace=_trace)
    out = np.empty((E_TOT, HID), dtype=f)
    for k in range(N_CORES):
        s2e = slot_edge[k]
        valid = s2e >= 0
        out[s2e[valid]] = res.results[k]["t_out"][:, valid].T
    if _trace:
        kernel._last_results = res
    return out


# revision 43
# speedup vs baseline: 1.5439x; 1.3088x over previous
"""Trainium2 Bass kernel for nn_Attr_Relation_Net (gnn_message_passing).

Computation per edge e (E = 400000):
    obs_h  = obs_embs[obs_idx[e]]                 # [256] gather
    m_i    = known_mask[obs_mask_idx[e]]          # [64]  gather
    a      = attr_idx[e]
    a_j_i  = G[a]   where G = feature_emb @ feature_emb.T   (64x64, on device)
    m      = m_i with column a zeroed             # m_i * self_mask[a]
    s      = softmax(m) = (1 + (e-1)*m) / (64 + (e-1)*sum(m))   (m in {0,1})
    mJI    = gelu(gelu(s @ rm_W1 + rm_b1) @ rm_W2 + rm_b2)
    h2     = gelu((a_j_i * mJI) @ rr_W + rr_b)
    out[e] = gelu((obs_h * h2) @ rc_W + rc_b)

Sharding: edges are assigned to the 8 cores by obs_idx range (12500 rows of
obs_embs per core, so the obs_embs table is sharded by row and gather indices
fit int16 for the fast bulk-gather ucode). Within a core, edges are bucketed
into 4 obs_mask_idx ranges (<=32768 rows each) occupying fixed slot regions,
so known_mask gathers read a sliced table view with int16-local indices.
The host assembles per-core slot orderings and un-permutes the output.

On-chip layout: all matmul stages run feat-major ([feat, edges]). obs_embs is
stored bf16 in DRAM and bulk-gathered with transpose=True so obs_h arrives
feat-major directly (no PE transposes). The per-edge front end (masking +
closed-form softmax) runs row-major where per-edge broadcasts and reductions
are native; one PE transpose per 128 edges moves [eq | s] into feat-major.
a_j_i = G @ eq on PE (bf16). The final matmul is weight-stationary
(lhsT = rc_W chunks) producing feat-major output; rc_b is folded into the
GELU activation's per-partition bias; the output is written to DRAM as
[HID, ECP] and transposed back on the host. All matmuls run in bf16 with
f32 PSUM accumulation (full PE stream rate, lowest PE power under the
board's 4-of-8 duty-cycle throttle). The emission order software-pipelines
tiles: front end of tile t+1 is emitted before the heavy phase of tile t.
"""

import numpy as np

try:
    import ml_dtypes
    BF16 = np.dtype(ml_dtypes.bfloat16)
except ImportError:  # pragma: no cover
    BF16 = np.float32

E_TOT = 400000
N_CORES = 8
HID = 256
NF = 64
N_ROWS = 100000
P = 128

RPC = N_ROWS // N_CORES        # obs_embs rows per core (12500)
MASK_BASES = (0, 32768, 65536, 98304, 100000)
CAPQ = (17408, 17408, 17408, 1024)   # per-mask-range slot capacities
ECP = sum(CAPQ)                # 53248 slots per core
CHUNK = 2048                   # obs-gather granularity
MIBLK = 1024                   # known_mask-gather granularity
W = 512                        # compute tile (edges)
N_CHUNKS = ECP // CHUNK        # 26
TILES_PER_CHUNK = CHUNK // W   # 4
NGROUP = W // 128              # 4
NJ = ECP // P                  # 416 wrapped columns
NI16 = ECP // 16               # 3328 idx columns

EM1 = float(np.e - 1.0)

# region of each 1024-slot block
_CUM = np.cumsum((0,) + CAPQ)
BLK_REGION = [int(np.searchsorted(_CUM, b * MIBLK, side="right") - 1)
              for b in range(ECP // MIBLK)]


def build_nc(sim_gelu=False, n_chunks=N_CHUNKS):
    import concourse.bacc as bacc
    import concourse.mybir as mybir
    import concourse.tile as tile
    from concourse.masks import make_identity

    f32 = mybir.dt.float32
    f32r = mybir.dt.float32r
    bf16 = mybir.dt.bfloat16
    i16 = mybir.dt.int16
    i32 = mybir.dt.int32
    GELU = (mybir.ActivationFunctionType.Tanh if sim_gelu
            else mybir.ActivationFunctionType.Gelu)

    nc = bacc.Bacc("TRN2", target_bir_lowering=False, debug=False,
                   enable_asserts=True, num_devices=N_CORES)

    # --- DRAM tensors (per core) ---
    t_obsidx = nc.dram_tensor("t_obsidx", [P, NI16], i16, kind="ExternalInput").ap()
    t_omask = nc.dram_tensor("t_omask", [P, NI16], i16, kind="ExternalInput").ap()
    t_attrf = nc.dram_tensor("t_attrf", [P, NJ], f32, kind="ExternalInput").ap()
    t_kmask = nc.dram_tensor("t_kmask", [N_ROWS, NF], f32, kind="ExternalInput").ap()
    t_obs = nc.dram_tensor("t_obs", [RPC, HID], bf16, kind="ExternalInput").ap()
    t_femb = nc.dram_tensor("t_femb", [NF, HID], f32, kind="ExternalInput").ap()
    t_w1 = nc.dram_tensor("t_w1", [NF, HID], f32, kind="ExternalInput").ap()
    t_b1 = nc.dram_tensor("t_b1", [HID], f32, kind="ExternalInput").ap()
    t_w2 = nc.dram_tensor("t_w2", [HID, NF], f32, kind="ExternalInput").ap()
    t_b2 = nc.dram_tensor("t_b2", [NF], f32, kind="ExternalInput").ap()
    t_wr = nc.dram_tensor("t_wr", [NF, HID], f32, kind="ExternalInput").ap()
    t_br = nc.dram_tensor("t_br", [HID], f32, kind="ExternalInput").ap()
    t_wc = nc.dram_tensor("t_wc", [HID, HID], f32, kind="ExternalInput").ap()
    t_bc = nc.dram_tensor("t_bc", [HID], f32, kind="ExternalInput").ap()
    t_out = nc.dram_tensor("t_out", [HID, ECP], f32, kind="ExternalOutput").ap()

    with tile.TileContext(nc) as tc:
        with tc.tile_pool(name="const", bufs=1) as cp, \
             tc.tile_pool(name="chunkp", bufs=2) as chp, \
             tc.tile_pool(name="work", bufs=2) as wp:

            # ---------- constants / weights ----------
            ident = cp.tile([P, P], f32)
            make_identity(nc, ident[:])
            identb = cp.tile([P, P], bf16)
            nc.vector.tensor_copy(out=identb[:], in_=ident[:])

            iota_i = cp.tile([P, NF], i32)
            nc.gpsimd.iota(iota_i[:], pattern=[[1, NF]], base=0, channel_multiplier=0)
            iota_f = cp.tile([P, NF], f32)
            nc.vector.tensor_copy(out=iota_f[:], in_=iota_i[:])

            # weights, cast to bf16 on chip (lower PE power, full stream rate)
            w1_sb = cp.tile([P, HID], bf16)         # rows 64:128 = rm_W1
            nc.gpsimd.dma_start(out=w1_sb[64:128, :], in_=t_w1[:])
            wstage = cp.tile([P, 2, HID], f32)
            nc.sync.dma_start(out=wstage[:, 0, 0:NF], in_=t_w2[0:128, :])
            nc.sync.dma_start(out=wstage[:, 1, 0:NF], in_=t_w2[128:256, :])
            w2_sb = cp.tile([P, 2, NF], bf16)       # [:,h,:] = rm_W2[128h:128h+128]
            nc.vector.tensor_copy(out=w2_sb[:], in_=wstage[:, :, 0:NF])
            wstage2 = cp.tile([NF, HID], f32)
            nc.sync.dma_start(out=wstage2[:], in_=t_wr[:])
            wr_sb = cp.tile([NF, HID], bf16)
            nc.vector.tensor_copy(out=wr_sb[:], in_=wstage2[:])
            wstage3 = cp.tile([P, 2, HID], f32)
            nc.sync.dma_start(out=wstage3[:, 0, :], in_=t_wc[0:128, :])
            nc.sync.dma_start(out=wstage3[:, 1, :], in_=t_wc[128:256, :])
            wc_sb = cp.tile([P, 2, HID], bf16)
            nc.vector.tensor_copy(out=wc_sb[:], in_=wstage3[:])

            # biases (per-partition columns)
            b1_sb = cp.tile([P, 2], f32)
            nc.sync.dma_start(out=b1_sb[:, 0:1], in_=t_b1[0:128, None])
            nc.sync.dma_start(out=b1_sb[:, 1:2], in_=t_b1[128:256, None])
            b2_sb = cp.tile([NF, 1], f32)
            nc.sync.dma_start(out=b2_sb[:], in_=t_b2[:, None])
            br_sb = cp.tile([P, 2], f32)
            nc.sync.dma_start(out=br_sb[:, 0:1], in_=t_br[0:128, None])
            nc.sync.dma_start(out=br_sb[:, 1:2], in_=t_br[128:256, None])
            bc_sb = cp.tile([P, 2], f32)
            nc.sync.dma_start(out=bc_sb[:, 0:1], in_=t_bc[0:128, None])
            nc.sync.dma_start(out=bc_sb[:, 1:2], in_=t_bc[128:256, None])

            # index arrays
            obsidx_sb = cp.tile([P, NI16], i16)
            nc.sync.dma_start(out=obsidx_sb[:], in_=t_obsidx[:])
            omask_sb = cp.tile([P, NI16], i16)
            nc.sync.dma_start(out=omask_sb[:], in_=t_omask[:])
            attr_sb = cp.tile([P, NJ], f32)
            nc.sync.dma_start(out=attr_sb[:], in_=t_attrf[:])

            # ---------- G = femb @ femb.T (bf16 for direct use with eqT) ---
            femb_sb = cp.tile([NF, HID], f32)
            nc.sync.dma_start(out=femb_sb[:], in_=t_femb[:])
            with tc.tile_pool(name="setup_ps", bufs=1, space="PSUM") as spp:
                ft_ps = spp.tile([P, 2, NF], f32, tag="setup")
                nc.tensor.transpose(out=ft_ps[:, 0, :], in_=femb_sb[:, 0:P],
                                    identity=ident[0:NF, 0:NF])
                nc.tensor.transpose(out=ft_ps[:, 1, :], in_=femb_sb[:, P:HID],
                                    identity=ident[0:NF, 0:NF])
                ft_sb = cp.tile([P, 2, NF], f32r)
                nc.vector.tensor_copy(out=ft_sb[:], in_=ft_ps[:])
                g_ps = spp.tile([NF, NF], f32, tag="setupg")
                nc.tensor.matmul(out=g_ps[:], lhsT=ft_sb[:, 0, :],
                                 rhs=ft_sb[:, 0, :], start=True, stop=False)
                nc.tensor.matmul(out=g_ps[:], lhsT=ft_sb[:, 1, :],
                                 rhs=ft_sb[:, 1, :], start=False, stop=True)
                g_sb = cp.tile([NF, NF], bf16)
                nc.vector.tensor_copy(out=g_sb[:], in_=g_ps[:])

            # ---------- main loop (1-tile software-pipeline skew) ----------
            _pp_cm = tc.tile_pool(name="psum", bufs=1, space="PSUM")
            pp = _pp_cm.__enter__()
            n_tiles = n_chunks * TILES_PER_CHUNK
            mi_bl = {}      # chunk -> [mi_t0, mi_t1]
            obsT_h = {}     # chunk -> obsT tile
            seqT_h = {}     # tile  -> seqT tile

            def emit_gathers(c):
                # known_mask gathers: two 1024-row blocks, each within one
                # mask-range region (sliced table keeps indices int16)
                bl = []
                for hb in range(2):
                    b = 2 * c + hb
                    q = BLK_REGION[b]
                    mi_t = chp.tile([P, MIBLK // P, NF], f32, tag=f"mi{hb}",
                                    name=f"mi_t{hb}", bufs=3)
                    nc.gpsimd.dma_gather(
                        out_ap=mi_t[:],
                        in_ap=t_kmask[MASK_BASES[q]:MASK_BASES[q + 1], :],
                        idxs_ap=omask_sb[:, b * (MIBLK // 16):
                                         (b + 1) * (MIBLK // 16)],
                        num_idxs=MIBLK, num_idxs_reg=MIBLK, elem_size=NF,
                        single_packet=False,
                    )
                    bl.append(mi_t)
                mi_bl[c] = bl
                # obs_h gather: 2048 rows of [256] bf16, transposed on the
                # fly into feat-major [128, 2, 2048]
                obsT = chp.tile([P, 2, CHUNK], bf16, tag="obs", bufs=3)
                nc.gpsimd.dma_gather(
                    out_ap=obsT[:], in_ap=t_obs[:],
                    idxs_ap=obsidx_sb[:, c * (CHUNK // 16):
                                      (c + 1) * (CHUNK // 16)],
                    num_idxs=CHUNK, num_idxs_reg=CHUNK, elem_size=HID,
                    transpose=True, single_packet=False,
                )
                obsT_h[c] = obsT

            def emit_fe(t):
                # front end (row-major) + transpose to feat-major
                c, sti = t // TILES_PER_CHUNK, t % TILES_PER_CHUNK
                attr_v = attr_sb[:, t * NGROUP:(t + 1) * NGROUP]
                mi_v = mi_bl[c][sti // 2][:, (sti % 2) * NGROUP:
                                          (sti % 2 + 1) * NGROUP, :]

                stin = wp.tile([P, NGROUP, 2, NF], bf16, tag="stin")
                noteq = wp.tile([P, NGROUP, NF], f32, tag="noteq")
                nc.vector.tensor_tensor(
                    out=noteq[:],
                    in0=attr_v.unsqueeze(2).broadcast_to([P, NGROUP, NF]),
                    in1=iota_f[:].unsqueeze(1).broadcast_to([P, NGROUP, NF]),
                    op=mybir.AluOpType.not_equal,
                )
                # eq = 1 - noteq  -> transpose staging cols 0:64.
                # On the scalar engine: its SBUF port is not starved by
                # concurrent gathers, unlike DVE writes.
                nc.scalar.activation(
                    out=stin[:, :, 0, :], in_=noteq[:],
                    func=mybir.ActivationFunctionType.Copy,
                    bias=1.0, scale=-1.0,
                )
                m_sb = wp.tile([P, NGROUP, NF], f32, tag="m")
                nc.vector.tensor_tensor(
                    out=m_sb[:], in0=noteq[:], in1=mi_v,
                    op=mybir.AluOpType.mult,
                )
                n1 = wp.tile([P, NGROUP], f32, tag="n1")
                nc.vector.tensor_reduce(out=n1[:], in_=m_sb[:],
                                        axis=mybir.AxisListType.X,
                                        op=mybir.AluOpType.add)
                dden = wp.tile([P, NGROUP], f32, tag="dden")
                nc.scalar.activation(
                    out=dden[:], in_=n1[:],
                    func=mybir.ActivationFunctionType.Copy,
                    bias=float(NF), scale=EM1,
                )
                rr = wp.tile([P, NGROUP], f32, tag="rr")
                nc.vector.reciprocal_approx_fast(out=rr[:], in_=dden[:])
                rr2 = wp.tile([P, NGROUP], f32, tag="rr2")
                nc.scalar.activation(
                    out=rr2[:], in_=rr[:],
                    func=mybir.ActivationFunctionType.Copy,
                    bias=0.0, scale=EM1,
                )
                # s = (1 + EM1*m)*rr = m*(EM1*rr) + rr: per-group scalar
                # activations with per-partition scale/bias APs (keeps the
                # stall-prone strided writes off the DVE)
                for g in range(NGROUP):
                    nc.scalar.activation(
                        out=stin[:, g, 1, :], in_=m_sb[:, g, :],
                        func=mybir.ActivationFunctionType.Identity,
                        bias=rr[:, g:g + 1], scale=rr2[:, g:g + 1],
                    )

                stinT_ps = pp.tile([P, NGROUP, P], bf16, tag="xps", bufs=2)
                for g in range(NGROUP):
                    nc.tensor.transpose(
                        out=stinT_ps[:, g, :],
                        in_=stin[:, g, :, :].rearrange("p a f -> p (a f)"),
                        identity=identb[:])
                seqT = wp.tile([P, W], bf16, tag="seqT")
                nc.scalar.activation(
                    out=seqT[:].rearrange("p (g e) -> p g e", g=NGROUP),
                    in_=stinT_ps[:],
                    func=mybir.ActivationFunctionType.Copy, scale=1.0)
                seqT_h[t] = seqT

            def emit_heavy(t):
                c, sti = t // TILES_PER_CHUNK, t % TILES_PER_CHUNK
                seqT = seqT_h.pop(t)
                eqT = seqT[0:NF, :]          # partitions 0:64 (bf16)
                sT = seqT[NF:P, :]           # partitions 64:128

                # aji^T = G @ eqT (bf16)
                aji_t = pp.tile([NF, W], f32, tag="aji")
                aji_ps = aji_t[:]
                nc.tensor.matmul(out=aji_ps, lhsT=g_sb[:],
                                 rhs=eqT, start=True, stop=True)

                # h1 = gelu(W1^T s + b1)   [256, W] in one 2-bank psum
                h1_ps = pp.tile([P, 2, W], f32, tag="h1h2")
                for h in range(2):
                    nc.tensor.matmul(out=h1_ps[:, h, :],
                                     lhsT=w1_sb[64:128, h * P:(h + 1) * P],
                                     rhs=sT, start=True, stop=True)
                h1T = wp.tile([P, 2, W], bf16, tag="h1T")
                for h in range(2):
                    nc.scalar.activation(out=h1T[:, h, :], in_=h1_ps[:, h, :],
                                         func=GELU, bias=b1_sb[:, h:h + 1],
                                         scale=1.0)

                # mJI = gelu(W2^T h1 + b2)  [64, W]
                mji_t = pp.tile([NF, W], f32, tag="mji")
                mji_ps = mji_t[:]
                nc.tensor.matmul(out=mji_ps, lhsT=w2_sb[:, 0, :],
                                 rhs=h1T[:, 0, :], start=True, stop=False)
                nc.tensor.matmul(out=mji_ps, lhsT=w2_sb[:, 1, :],
                                 rhs=h1T[:, 1, :], start=False, stop=True)
                mjiT = wp.tile([NF, W], f32, tag="mjiT")
                nc.scalar.activation(out=mjiT[:], in_=mji_ps,
                                     func=GELU, bias=b2_sb[:, 0:1], scale=1.0)

                # u = mJI * aji   [64, W]
                u_sb = wp.tile([NF, W], bf16, tag="u")
                nc.vector.tensor_tensor(out=u_sb[:], in0=mjiT[:],
                                        in1=aji_ps,
                                        op=mybir.AluOpType.mult)

                # h2 = gelu(Wr^T u + br)  [256, W]
                h2_ps = pp.tile([P, 2, W], f32, tag="h1h2")
                for h in range(2):
                    nc.tensor.matmul(out=h2_ps[:, h, :],
                                     lhsT=wr_sb[:, h * P:(h + 1) * P],
                                     rhs=u_sb[:], start=True, stop=True)
                h2T = wp.tile([P, 2, W], f32, tag="h2T")
                for h in range(2):
                    nc.scalar.activation(out=h2T[:, h, :], in_=h2_ps[:, h, :],
                                         func=GELU, bias=br_sb[:, h:h + 1],
                                         scale=1.0)

                # v = obs_h^T * h2 (obs_h already feat-major)
                vT = wp.tile([P, 2, W], bf16, tag="vT")
                nc.vector.tensor_tensor(
                    out=vT[:],
                    in0=obsT_h[c][:, :, sti * W:(sti + 1) * W],
                    in1=h2T[:], op=mybir.AluOpType.mult)

                # out^T = Wc^T v, feat-major [256, W]
                out_ps = pp.tile([P, 2, W], f32, tag="out")
                for mc in range(2):
                    for h in range(2):
                        nc.tensor.matmul(
                            out=out_ps[:, mc, :],
                            lhsT=wc_sb[:, h, mc * P:(mc + 1) * P],
                            rhs=vT[:, h, :],
                            start=(h == 0), stop=(h == 1))

                out_sb = wp.tile([P, 2, W], f32, tag="outsb")
                for mc in range(2):
                    nc.scalar.activation(
                        out=out_sb[:, mc, :], in_=out_ps[:, mc, :],
                        func=GELU, bias=bc_sb[:, mc:mc + 1], scale=1.0)

                dst = t_out[:, t * W:(t + 1) * W].rearrange(
                    "(c p) w -> p c w", p=P)
                nc.sync.dma_start(out=dst, in_=out_sb[:])

            # 1-tile software-pipeline skew: tile t+1's front end is
            # emitted before tile t's heavy phase.
            emit_gathers(0)
            emit_fe(0)
            for t in range(n_tiles):
                nt = t + 1
                if nt < n_tiles:
                    if nt % TILES_PER_CHUNK == 0:
                        emit_gathers(nt // TILES_PER_CHUNK)
                    emit_fe(nt)
                emit_heavy(t)
            _pp_cm.__exit__(None, None, None)

    nc.compile()
    return nc


_NC_CACHE = {}


def _get_nc(sim_gelu=False, n_chunks=N_CHUNKS):
    key = (bool(sim_gelu), n_chunks)
    if key not in _NC_CACHE:
        _NC_CACHE[key] = build_nc(sim_gelu=key[0], n_chunks=key[1])
    return _NC_CACHE[key]


def _wrap16(a):
    """[ECP] int16 -> [128, ECP//16]: idx j at [j%16, j//16], replicated 8x
    across partition groups (one copy per Q7 core)."""
    w = np.ascontiguousarray(a.reshape(ECP // 16, 16).T)
    return np.ascontiguousarray(np.tile(w, (8, 1)))


def _wrapP(a, dtype):
    """[ECP] -> [128, ECP//128]: slot j*128+p at [p, j]."""
    return np.ascontiguousarray(a.astype(dtype).reshape(NJ, P).T)


def make_in_maps(known_mask, obs_idx, obs_mask_idx, attr_idx, obs_embs,
                 feature_emb, weights):
    """Bucket edges by (core = obs_idx // 12500, region = mask range), build
    per-core marshalled inputs. Returns (in_maps, slot_edge[8])."""
    f = np.float32
    obs_idx = np.asarray(obs_idx).ravel().astype(np.int64)
    obs_mask_idx = np.asarray(obs_mask_idx).ravel().astype(np.int64)
    attr_idx = np.asarray(attr_idx).ravel().astype(np.int64)

    known_mask = np.ascontiguousarray(known_mask, dtype=f)
    obs_embs_bf = np.ascontiguousarray(np.asarray(obs_embs, dtype=f).astype(BF16))
    feature_emb = np.ascontiguousarray(feature_emb, dtype=f)

    core_of = obs_idx // RPC
    region_of = np.searchsorted(MASK_BASES, obs_mask_idx, side="right") - 1

    in_maps = []
    slot_edge = []
    for k in range(N_CORES):
        loc_obs = np.zeros(ECP, np.int16)
        loc_msk = np.zeros(ECP, np.int16)
        loc_atr = np.zeros(ECP, f)
        s2e = np.full(ECP, -1, np.int64)
        base = 0
        for q in range(4):
            sel = np.nonzero((core_of == k) & (region_of == q))[0]
            n = sel.shape[0]
            if n > CAPQ[q]:
                raise RuntimeError(
                    f"bucket overflow core={k} region={q}: {n} > {CAPQ[q]}")
            sl = slice(base, base + n)
            loc_obs[sl] = (obs_idx[sel] - k * RPC).astype(np.int16)
            loc_msk[sl] = (obs_mask_idx[sel] - MASK_BASES[q]).astype(np.int16)
            loc_atr[sl] = attr_idx[sel].astype(f)
            s2e[sl] = sel
            base += CAPQ[q]
        in_maps.append({
            "t_obsidx": _wrap16(loc_obs),
            "t_omask": _wrap16(loc_msk),
            "t_attrf": _wrapP(loc_atr, f),
            "t_kmask": known_mask,
            "t_obs": np.ascontiguousarray(obs_embs_bf[k * RPC:(k + 1) * RPC]),
            "t_femb": feature_emb,
            **weights,
        })
        slot_edge.append(s2e)
    return in_maps, slot_edge


def kernel(known_mask, obs_idx, obs_mask_idx, attr_idx_need_to_be_impute,
           obs_embs, feature_emb,
           rm_W1, rm_b1, rm_W2, rm_b2, rr_W, rr_b, rc_W, rc_b,
           _sim_gelu=False, _trace=False):
    from concourse.bass_utils import run_bass_kernel_spmd

    f = np.float32
    weights = {
        "t_w1": np.ascontiguousarray(rm_W1, dtype=f),
        "t_b1": np.ascontiguousarray(rm_b1, dtype=f),
        "t_w2": np.ascontiguousarray(rm_W2, dtype=f),
        "t_b2": np.ascontiguousarray(rm_b2, dtype=f),
        "t_wr": np.ascontiguousarray(rr_W, dtype=f),
        "t_br": np.ascontiguousarray(rr_b, dtype=f),
        "t_wc": np.ascontiguousarray(rc_W, dtype=f),
        "t_bc": np.ascontiguousarray(rc_b, dtype=f),
    }
    in_maps, slot_edge = make_in_maps(
        known_mask, obs_idx, obs_mask_idx, attr_idx_need_to_be_impute,
        obs_embs, feature_emb, weights)

    nc = _get_nc(sim_gelu=_sim_gelu)
    res = run_bass_kernel_spmd(nc, in_maps, core_ids=list(range(N_CORES)),
                               trace=_trace)
    out = np.empty((E_TOT, HID), dtype=f)
    for k in range(N_CORES):
        s2e = slot_edge[k]
        valid = s2e >= 0
        out[s2e[valid]] = res.results[k]["t_out"][:, valid].T
    if _trace:
        kernel._last_results = res
    return out
